# revision 24
# baseline (speedup 1.0000x reference)
"""Trainium2 Bass kernel for nn_DetectMultiImage (YOLO-style box decode + compaction).

Contract: kernel(output, confidence_threshold) takes the FULL [64,18,160,160] f32
feature map, returns the FULL [4915200, 6] f32 boxes tensor (valid detections
first in row order, zero rows after), matching the jax reference.

Strategy: pure data parallel over the batch axis — 8 images per NeuronCore.
The default impl ("raw16b", hand-scheduled raw bass) moves fp16 instead of f32
(12.3MB per core vs 24.6MB — the kernel is chip-HBM-bandwidth-bound, so bytes
are the whole game; the 2e-2 harness tolerance leaves fp16 plenty of margin):

- Host packs the 15 useful channels per image into [P=128, 3000] fp16 tiles:
  a 1800-col tanh group (f1,f2,f5 x 3 anchors) and a 1200-col exp group
  (f3,f4) with ln(anchor_w/h) pre-added so both exp fields run as ONE
  unbiased ACT op. All DMA descriptors are contiguous 6000B/partition runs.
- Per image the device runs 2 ACT ops (tanh->tmp, exp->out) and 3 DVE
  scalar_tensor_tensor ops (out_plane = tanh*k + grid_const). Sigmoid is
  0.5 + 0.5*tanh(x/2) (tanh and exp share the exp_and_others ACT table; no
  table switches). Grid constants are generated on device by Pool iota + DVE
  arithmetic during the input ramp (no const DMA).
- Engine split: sync=HWDGE input DMAs, scalar=ACT, vector=DVE, gpsimd=SWDGE
  output DMAs, cyclic buffers (in x5 / out x4 / tmp x2) with cumulative
  semaphore thresholds. Order-sensitive waits get one semaphore per DMA slot.
  The out plane layout is (cx, cy, th, w, h) x (anchor-major 600).
- Host does what it must do anyway for compaction (sigmoid over the conf
  logits + the stable valid-first mask), plus the (p, a, j) -> row-order
  permute of the device planes and an exact recompute of the ~2k rows where
  fp16 tanh near -1 cancels against the +1 grid/anchor offset (ix=0 / iy=0 /
  a=0 with logit < -3).

Measured on trn2 (8 cores, NTFF profile of core 0): 43.8-48.9us (median ~44.2)
vs the 85-95us f32 baseline; the data window runs with zero DMA idle at
318-376GB/s/core, i.e. at the chip HBM roofline (variance = cross-core HBM
contention). Exec = ~8.7us fixed program preamble (instruction prefetch,
barriers, queue-reg loads) + data window + ~1.8us tail (DMA sem prop + exit
barrier, with the SWDGE dge_drain overlapped into the last transfers).
"""

import numpy as np

# Problem shape (hardcoded per harness contract)
N, C, H, W = 64, 18, 160, 160
A = 3                     # anchors
F = 6                     # fields per anchor: conf, cx, cy, w, h, theta
NCORES = 8
M = N // NCORES           # images per core
S = H * W                 # 25600 spatial positions
P = 128                   # SBUF partitions
J = S // P                # 200 spatial positions per partition per channel
CELL = 32.0
ANCHOR_W = 85.72
ANCHOR_H = 19.15
THETA_MARGIN = 60.0       # 180 / A

_nc_cache = {}

_LN_W = float(np.log(np.float32(ANCHOR_W)))
_LN_H = float(np.log(np.float32(ANCHOR_H)))


def _build_nc16():
    """fp16 I/O variant: per-core traffic drops to 12.3MB (vs 24.6MB f32).

    Host packs the 15 useful channels per image into a [P, 3000] fp16 tile:
    cols [0:1800) = tanh group (f1,f2,f5 x 3 anchors, 200 spatial each),
    cols [1800:3000) = exp group (f3,f4 x 3 anchors) with ln(anchor) pre-added
    so both exp fields fuse into one unbiased ACT op. Every DMA descriptor is
    a contiguous 6000B/partition run (>=512B, full DMA bus efficiency).

    Per image: 2 ACT ops (tanh 1800, exp 1200 -> out direct), 3 DVE
    scalar_tensor_tensor ops (out = tanh*k + const plane). Out tile [P, 3000]
    fp16 planes (cx, cy, th, w, h), each 600 cols a-major; host permutes
    (a, j) -> (j, a) row order during the compaction gather and patches the
    ~2k rows where fp16 tanh cancellation near -1 loses precision
    (ix=0 / iy=0 / a=0 with very negative logits).

    Input DMAs ride the sync/HWDGE ring; output DMAs issue from the Pool
    engine (SWDGE) so a not-yet-ready output never blocks input prefetch.
    """
    import concourse.bacc as bacc
    import concourse.mybir as mybir
    import concourse.tile as tile

    f16 = mybir.dt.float16
    AF = mybir.ActivationFunctionType
    ALU = mybir.AluOpType

    TG = 9 * J            # 1800 tanh-group cols
    EG = 6 * J            # 1200 exp-group cols
    TC = TG + EG          # 3000

    nc = bacc.Bacc("TRN2", target_bir_lowering=False, debug=False)

    x = nc.dram_tensor("x", [M, P, TC], f16, kind="ExternalInput")
    cc = nc.dram_tensor("cc", [P, TG], f16, kind="ExternalInput")
    y = nc.dram_tensor("y", [M, P, TC], f16, kind="ExternalOutput")

    with tile.TileContext(nc) as tc:
        with (
            tc.tile_pool(name="const", bufs=1) as constp,
            tc.tile_pool(name="inp", bufs=4) as inp,
            tc.tile_pool(name="outp", bufs=3) as outp,
            tc.tile_pool(name="tmp", bufs=3) as tmpp,
        ):
            cc_t = constp.tile([P, TG], f16, tag="cc")
            nc.sync.dma_start(cc_t[:], cc.ap())

            SC = (16.0, 16.0, 30.0)
            for n in range(M):
                in_t = inp.tile([P, TC], f16, tag="in")
                if n == 0:
                    # ramp: land the tanh group first so ACT starts early
                    nc.sync.dma_start(in_t[:, 0:TG], x.ap()[n][:, 0:TG])
                    nc.sync.dma_start(in_t[:, TG:TC], x.ap()[n][:, TG:TC])
                else:
                    nc.sync.dma_start(in_t[:], x.ap()[n])

                t_t = tmpp.tile([P, TG], f16, tag="t")
                out_t = outp.tile([P, TC], f16, tag="out")
                nc.scalar.activation(t_t[:], in_t[:, 0:TG], AF.Tanh, scale=0.5)
                nc.scalar.activation(out_t[:, TG:TC], in_t[:, TG:TC], AF.Exp)
                # w/h planes ready first: ship them without waiting on DVE
                nc.gpsimd.dma_start(y.ap()[n][:, TG:TC], out_t[:, TG:TC])
                for k in range(3):
                    a0, a1 = k * A * J, (k + 1) * A * J
                    nc.vector.scalar_tensor_tensor(
                        out_t[:, a0:a1], t_t[:, a0:a1], SC[k],
                        cc_t[:, a0:a1], ALU.mult, ALU.add,
                    )
                nc.gpsimd.dma_start(y.ap()[n][:, 0:TG], out_t[:, 0:TG])

    nc.compile()
    return nc


def _build_nc_raw16():
    """Hand-scheduled raw-bass version of the fp16 kernel (same math/layout
    as _build_nc16) — drops the TileContext preamble barriers/queue-reg loads
    and per-queue exit drains, which cost ~10us of the 47.6us tile-mode exec.

    Engine split: sync issues input DMAs (HWDGE), scalar runs the 2 ACT ops
    per image + the const DMA on its own ring, vector the 3 stt ops, gpsimd
    (SWDGE) the output DMAs. Cyclic buffers in x4 / out x3 / tanh-tmp x2
    guarded by cumulative semaphore thresholds; DMA completions inc by 16.
    """
    from contextlib import ExitStack

    import concourse.bass as bass
    import concourse.mybir as mybir

    f16 = mybir.dt.float16
    f32 = mybir.dt.float32
    AF = mybir.ActivationFunctionType
    ALU = mybir.AluOpType

    TG = 9 * J
    EG = 6 * J
    TC = TG + EG

    nc = bass.Bass("TRN2", target_bir_lowering=False, debug=False)
    x = nc.dram_tensor("x", [M, P, TC], f16, kind="ExternalInput")
    cc = nc.dram_tensor("cc", [P, TG], f16, kind="ExternalInput")
    y = nc.dram_tensor("y", [M, P, TC], f16, kind="ExternalOutput")

    NBI, NBO, NBT = 4, 3, 2
    SC = (16.0, 16.0, 30.0)

    with ExitStack() as ctx:
        in_t = [ctx.enter_context(nc.sbuf_tensor(f"in{i}", [P, TC], f16))
                for i in range(NBI)]
        out_t = [ctx.enter_context(nc.sbuf_tensor(f"out{i}", [P, TC], f16))
                 for i in range(NBO)]
        t_t = [ctx.enter_context(nc.sbuf_tensor(f"t{i}", [P, TG], f16))
               for i in range(NBT)]
        cc_t = ctx.enter_context(nc.sbuf_tensor("cc_t", [P, TG], f16))
        s_cc = ctx.enter_context(nc.semaphore("s_cc"))
        s_ib = [ctx.enter_context(nc.semaphore(f"s_ib{i}")) for i in range(NBI)]
        s_ob = [ctx.enter_context(nc.semaphore(f"s_ob{i}")) for i in range(NBO)]
        s_act = ctx.enter_context(nc.semaphore("s_act"))
        s_dve = ctx.enter_context(nc.semaphore("s_dve"))
        block = ctx.enter_context(nc.Block())

        # cumulative semaphore thresholds (python-side bookkeeping)
        in_cum = [0] * NBI
        in_thr_t = {}   # s_ib[n%4] value once image n's tanh-group cols landed
        in_thr_e = {}   # ... once the full image landed
        for n in range(M):
            b = n % NBI
            if n == 0:   # split ramp DMA: tanh group first
                in_cum[b] += 16
                in_thr_t[n] = in_cum[b]
                in_cum[b] += 16
                in_thr_e[n] = in_cum[b]
            else:
                in_cum[b] += 16
                in_thr_t[n] = in_thr_e[n] = in_cum[b]
        ob_cum = [0] * NBO
        ob_before = {}  # s_ob[n%3] value before image n may write out buffer
        for n in range(M):
            b = n % NBO
            ob_before[n] = ob_cum[b]
            ndma = 3 if n == M - 1 else 2
            ob_cum[b] += 16 * ndma

        @block.sync
        def _(sync):
            for n in range(M):
                b = n % NBI
                if n >= NBI:  # WAR: ACT reads of image n-4 done
                    sync.wait_ge(s_act, 2 * (n - NBI + 1))
                if n == 0:
                    sync.dma_start(in_t[0].ap()[:, 0:TG],
                                   x.ap()[0][:, 0:TG]).then_inc(s_ib[0], 16)
                    sync.dma_start(in_t[0].ap()[:, TG:TC],
                                   x.ap()[0][:, TG:TC]).then_inc(s_ib[0], 16)
                else:
                    sync.dma_start(in_t[b].ap(),
                                   x.ap()[n]).then_inc(s_ib[b], 16)

        @block.scalar
        def _(scalar):
            # dummy ACTIVATE before any wait so the exp_and_others
            # ACT_TABLE_LOAD runs during the input ramp
            const0 = nc.const_aps.aps[(f32, 0.0)]
            nc.scalar.activation(t_t[0].ap()[:, 0:1], const0[:, 0:1], AF.Tanh)
            scalar.dma_start(cc_t.ap(), cc.ap()).then_inc(s_cc, 16)
            for n in range(M):
                b, ob, tb = n % NBI, n % NBO, n % NBT
                scalar.wait_ge(s_ib[b], in_thr_t[n])
                if n >= NBT:  # WAR on t tile vs DVE reads of image n-2
                    scalar.wait_ge(s_dve, 3 * (n - NBT + 1))
                nc.scalar.activation(
                    t_t[tb].ap(), in_t[b].ap()[:, 0:TG], AF.Tanh, scale=0.5,
                ).then_inc(s_act, 1)
                if n == 0:
                    scalar.wait_ge(s_ib[0], in_thr_e[0])
                if n >= NBO:  # WAR on out tile vs out DMAs of image n-3
                    scalar.wait_ge(s_ob[ob], ob_before[n])
                nc.scalar.activation(
                    out_t[ob].ap()[:, TG:TC], in_t[b].ap()[:, TG:TC], AF.Exp,
                ).then_inc(s_act, 1)

        @block.vector
        def _(vector):
            vector.wait_ge(s_cc, 16)
            for n in range(M):
                ob, tb = n % NBO, n % NBT
                vector.wait_ge(s_act, 2 * n + 1)  # tanh of image n done
                if n >= NBO:
                    vector.wait_ge(s_ob[ob], ob_before[n])
                for k in range(3):
                    a0, a1 = k * A * J, (k + 1) * A * J
                    nc.vector.scalar_tensor_tensor(
                        out_t[ob].ap()[:, a0:a1], t_t[tb].ap()[:, a0:a1],
                        SC[k], cc_t.ap()[:, a0:a1], ALU.mult, ALU.add,
                    ).then_inc(s_dve, 1)

        @block.gpsimd
        def _(gpsimd):
            for n in range(M):
                ob = n % NBO
                gpsimd.wait_ge(s_act, 2 * n + 2)  # exp done: ship w/h planes
                gpsimd.dma_start(y.ap()[n][:, TG:TC],
                                 out_t[ob].ap()[:, TG:TC]).then_inc(s_ob[ob], 16)
                if n == M - 1:  # shorter drain tail: split the last DVE planes
                    gpsimd.wait_ge(s_dve, 3 * n + 2)
                    gpsimd.dma_start(
                        y.ap()[n][:, 0:2 * A * J],
                        out_t[ob].ap()[:, 0:2 * A * J]).then_inc(s_ob[ob], 16)
                    gpsimd.wait_ge(s_dve, 3 * n + 3)
                    gpsimd.dma_start(
                        y.ap()[n][:, 2 * A * J:TG],
                        out_t[ob].ap()[:, 2 * A * J:TG]).then_inc(s_ob[ob], 16)
                else:
                    gpsimd.wait_ge(s_dve, 3 * (n + 1))
                    gpsimd.dma_start(
                        y.ap()[n][:, 0:TG],
                        out_t[ob].ap()[:, 0:TG]).then_inc(s_ob[ob], 16)
            for b2 in range(NBO):
                gpsimd.wait_ge(s_ob[b2], ob_cum[b2])

    return nc


def _build_nc_raw16b(nbi=5, nbo=4):
    """raw16 + deeper buffering (in x5, out x4) + device-generated grid
    constants: Pool/gpsimd computes c1/c2/cth via iota+mod during the input
    ramp (Pool is otherwise idle there), dropping the 460KB const DMA from
    the DMA-bound window. No host const tensor at all.
    """
    from contextlib import ExitStack

    import concourse.bass as bass
    import concourse.mybir as mybir

    f16 = mybir.dt.float16
    f32 = mybir.dt.float32
    i32 = mybir.dt.int32
    AF = mybir.ActivationFunctionType
    ALU = mybir.AluOpType

    TG = 9 * J
    EG = 6 * J
    TC = TG + EG

    nc = bass.Bass("TRN2", target_bir_lowering=False, debug=False)
    x = nc.dram_tensor("x", [M, P, TC], f16, kind="ExternalInput")
    y = nc.dram_tensor("y", [M, P, TC], f16, kind="ExternalOutput")

    NBI, NBO, NBT = nbi, nbo, 2
    SC = (16.0, 16.0, 30.0)

    with ExitStack() as ctx:
        in_t = [ctx.enter_context(nc.sbuf_tensor(f"in{i}", [P, TC], f16))
                for i in range(NBI)]
        out_t = [ctx.enter_context(nc.sbuf_tensor(f"out{i}", [P, TC], f16))
                 for i in range(NBO)]
        t_t = [ctx.enter_context(nc.sbuf_tensor(f"t{i}", [P, TG], f16))
               for i in range(NBT)]
        qs_t = ctx.enter_context(nc.sbuf_tensor("qs", [P, J], i32))
        qy_t = ctx.enter_context(nc.sbuf_tensor("qy", [P, J], f32))
        q0_t = ctx.enter_context(nc.sbuf_tensor("q0", [P, J], f32))
        c1_t = ctx.enter_context(nc.sbuf_tensor("c1", [P, J], f16))
        c2_t = ctx.enter_context(nc.sbuf_tensor("c2", [P, J], f16))
        ct_t = ctx.enter_context(nc.sbuf_tensor("ct", [P, A * J], f16))
        s_iot = ctx.enter_context(nc.semaphore("s_iot"))
        s_i0 = ctx.enter_context(nc.semaphore("s_i0"))
        s_ib = [ctx.enter_context(nc.semaphore(f"s_ib{i}")) for i in range(NBI)]
        s_ob = [ctx.enter_context(nc.semaphore(f"s_ob{i}")) for i in range(NBO)]
        s_act = ctx.enter_context(nc.semaphore("s_act"))
        s_dve = ctx.enter_context(nc.semaphore("s_dve"))
        # keep the full exit dge_drain: skipping it (no_gpsimd_drain=True)
        # saved ~1.5us but left the SWDGE unit undrained at NEFF exit, which
        # intermittently faulted the NEXT execution with
        # NRT_EXEC_UNIT_UNRECOVERABLE (observed ~1 in 10 runs). Instead the
        # final output-completion waits sit on the idle sync engine, so the
        # drain overlaps the in-flight tail transfers (same ~1.5us back,
        # without skipping the quiesce).
        block = ctx.enter_context(nc.Block())

        # image 0's split input lands on two sems (s_i0 for the tanh group,
        # s_ib[0] for the exp group) so neither wait can be satisfied by the
        # other DMA completing first
        in_cum = [0] * NBI
        in_thr = {}
        for n in range(M):
            b = n % NBI
            in_cum[b] += 16
            in_thr[n] = in_cum[b]
        ob_cum = [0] * NBO
        ob_before = {}
        for n in range(M):
            b = n % NBO
            ob_before[n] = ob_cum[b]
            ndma = 3 if n == M - 1 else 2
            ob_cum[b] += 16 * ndma

        # broadcast views for the cx/cy in1 operands (one [P,J] column tile
        # replicated over the anchor dim with stride 0)
        c1v = c1_t.ap().unsqueeze(1).broadcast_to([P, A, J])
        c2v = c2_t.ap().unsqueeze(1).broadcast_to([P, A, J])

        @block.sync
        def _(sync):
            for n in range(M):
                b = n % NBI
                if n >= NBI:
                    sync.wait_ge(s_act, 2 * (n - NBI + 1))
                if n == 0:
                    sync.dma_start(in_t[0].ap()[:, 0:TG],
                                   x.ap()[0][:, 0:TG]).then_inc(s_i0, 16)
                    sync.dma_start(in_t[0].ap()[:, TG:TC],
                                   x.ap()[0][:, TG:TC]).then_inc(s_ib[0], 16)
                else:
                    sync.dma_start(in_t[b].ap(),
                                   x.ap()[n]).then_inc(s_ib[b], 16)
            # final output-completion waits live here (sync idles at the end)
            # rather than on gpsimd, so gpsimd reaches the Block exit right
            # after issuing the last DMA and its dge_drain overlaps the
            # in-flight transfers; the exit barrier still can't pass until
            # these waits prove every output byte (and its sem) landed
            for b2 in range(NBO):
                sync.wait_ge(s_ob[b2], ob_cum[b2])

        @block.scalar
        def _(scalar):
            const0 = nc.const_aps.aps[(f32, 0.0)]
            nc.scalar.activation(t_t[0].ap()[:, 0:1], const0[:, 0:1], AF.Tanh)
            for n in range(M):
                b, ob, tb = n % NBI, n % NBO, n % NBT
                if n == 0:
                    scalar.wait_ge(s_i0, 16)
                else:
                    scalar.wait_ge(s_ib[b], in_thr[n])
                if n >= NBT:
                    scalar.wait_ge(s_dve, 3 * (n - NBT + 1))
                nc.scalar.activation(
                    t_t[tb].ap(), in_t[b].ap()[:, 0:TG], AF.Tanh, scale=0.5,
                ).then_inc(s_act, 1)
                if n == 0:
                    scalar.wait_ge(s_ib[0], in_thr[0])
                if n >= NBO:
                    scalar.wait_ge(s_ob[ob], ob_before[n])
                nc.scalar.activation(
                    out_t[ob].ap()[:, TG:TC], in_t[b].ap()[:, TG:TC], AF.Exp,
                ).then_inc(s_act, 1)

        @block.vector
        def _(vector):
            # grid constants from the Pool iota (s = 200p + j), add/mult only:
            # iy = round((s - 79.5)/160) via the 2^23 round-to-int trick,
            # c2 = 32*iy + 16, c1 = 32*ix + 16 = (32*s + 16) - 5120*iy,
            # cth = 60a + 30 via three memsets.
            # 1.5*2^23: force round-to-integer at the f32 SBUF write (ulp=1
            # there); each op carries a sem inc so no pass can fold the chain
            TWO23 = 12582912.0
            vector.wait_ge(s_iot, 1)
            nc.vector.tensor_scalar(out=qy_t.ap(), in0=qs_t.ap(),
                                    scalar1=-79.5, scalar2=1.0 / W,
                                    op0=ALU.add, op1=ALU.mult).then_inc(s_iot, 1)
            nc.vector.tensor_scalar(out=q0_t.ap(), in0=qy_t.ap(),
                                    scalar1=TWO23, scalar2=None,
                                    op0=ALU.add).then_inc(s_iot, 1)
            nc.vector.tensor_scalar(out=qy_t.ap(), in0=q0_t.ap(),
                                    scalar1=-TWO23, scalar2=None,
                                    op0=ALU.add).then_inc(s_iot, 1)
            nc.vector.tensor_scalar(out=c2_t.ap(), in0=qy_t.ap(),
                                    scalar1=32.0, scalar2=16.0,
                                    op0=ALU.mult, op1=ALU.add).then_inc(s_iot, 1)
            nc.vector.tensor_scalar(out=q0_t.ap(), in0=qs_t.ap(),
                                    scalar1=32.0, scalar2=16.0,
                                    op0=ALU.mult, op1=ALU.add).then_inc(s_iot, 1)
            nc.vector.scalar_tensor_tensor(
                c1_t.ap(), qy_t.ap(), -32.0 * W, q0_t.ap(),
                ALU.mult, ALU.add).then_inc(s_iot, 1)
            for k in range(A):
                nc.vector.memset(ct_t.ap()[:, k * J:(k + 1) * J], 60.0 * k + 30.0)
            for n in range(M):
                ob, tb = n % NBO, n % NBT
                vector.wait_ge(s_act, 2 * n + 1)
                if n >= NBO:
                    vector.wait_ge(s_ob[ob], ob_before[n])
                ov = out_t[ob].ap()
                tv = t_t[tb].ap()
                nc.vector.scalar_tensor_tensor(
                    ov[:, 0:A * J].rearrange("p (a j) -> p a j", a=A),
                    tv[:, 0:A * J].rearrange("p (a j) -> p a j", a=A),
                    SC[0], c1v, ALU.mult, ALU.add,
                ).then_inc(s_dve, 1)
                nc.vector.scalar_tensor_tensor(
                    ov[:, A * J:2 * A * J].rearrange("p (a j) -> p a j", a=A),
                    tv[:, A * J:2 * A * J].rearrange("p (a j) -> p a j", a=A),
                    SC[1], c2v, ALU.mult, ALU.add,
                ).then_inc(s_dve, 1)
                nc.vector.scalar_tensor_tensor(
                    ov[:, 2 * A * J:TG], tv[:, 2 * A * J:TG],
                    SC[2], ct_t.ap(), ALU.mult, ALU.add,
                ).then_inc(s_dve, 1)

        @block.gpsimd
        def _(gpsimd):
            nc.gpsimd.iota(qs_t.ap(), [[1, J]], base=0,
                           channel_multiplier=J).then_inc(s_iot, 1)
            for n in range(M):
                ob = n % NBO
                gpsimd.wait_ge(s_act, 2 * n + 2)
                gpsimd.dma_start(y.ap()[n][:, TG:TC],
                                 out_t[ob].ap()[:, TG:TC]).then_inc(s_ob[ob], 16)
                if n == M - 1:
                    gpsimd.wait_ge(s_dve, 3 * n + 2)
                    gpsimd.dma_start(
                        y.ap()[n][:, 0:2 * A * J],
                        out_t[ob].ap()[:, 0:2 * A * J]).then_inc(s_ob[ob], 16)
                    gpsimd.wait_ge(s_dve, 3 * n + 3)
                    gpsimd.dma_start(
                        y.ap()[n][:, 2 * A * J:TG],
                        out_t[ob].ap()[:, 2 * A * J:TG]).then_inc(s_ob[ob], 16)
                else:
                    gpsimd.wait_ge(s_dve, 3 * (n + 1))
                    gpsimd.dma_start(
                        y.ap()[n][:, 0:TG],
                        out_t[ob].ap()[:, 0:TG]).then_inc(s_ob[ob], 16)

    return nc


def _pack_inputs16(x):
    """[N,C,H,W] f32 -> [NCORES, M, P, 3000] fp16 device layout."""
    xs = x.reshape(NCORES, M, C, P, J)
    CH = [1, 7, 13, 2, 8, 14, 5, 11, 17, 3, 9, 15, 4, 10, 16]
    arr = xs[:, :, CH].transpose(0, 1, 3, 2, 4)      # [8, M, P, 15, J]
    bias = np.zeros((15, 1), np.float32)
    bias[9:12] = _LN_W
    bias[12:15] = _LN_H
    packed = (arr + bias).astype(np.float16)
    return np.ascontiguousarray(packed.reshape(NCORES, M, P, 15 * J))


def _const16():
    s = np.arange(S, dtype=np.int64).reshape(P, J)
    ix = (s % W).astype(np.float32)
    iy = (s // W).astype(np.float32)
    c1s = np.broadcast_to((32 * ix + 16)[:, None, :], (P, A, J))
    c2s = np.broadcast_to((32 * iy + 16)[:, None, :], (P, A, J))
    cth = np.broadcast_to(
        (60 * np.arange(A, dtype=np.float32) + 30)[None, :, None], (P, A, J)
    )
    cc = np.concatenate(
        [c1s.reshape(P, A * J), c2s.reshape(P, A * J), cth.reshape(P, A * J)],
        axis=1,
    ).astype(np.float16)
    return np.ascontiguousarray(cc)


_PATCH_THR = np.float32(-3.0)


def _sig(v):
    return np.float32(1.0) / (np.float32(1.0) + np.exp(-v))


def _unpack16(x, thr, results):
    """Device planes -> full [N*S*A, 6] f32 boxes with stable compaction."""
    yb = np.stack([np.asarray(r["y"]) for r in results])     # [8,M,P,3000] f16
    # [8, M, P, plane(5), a, j] -> row order (n, p, j, a) x field
    v = yb.reshape(NCORES, M, P, 5, A, J).transpose(0, 1, 2, 5, 4, 3)
    boxes5 = np.ascontiguousarray(v).reshape(N * S * A, 5).astype(np.float32)
    # plane order: 0=cx, 1=cy, 2=theta, 3=w, 4=h

    # patch rows where fp16 tanh near -1 cancels against the +1 grid/anchor
    # offset (ix=0 / iy=0 / a=0 with logit < -3): recompute exactly on host.
    for a in range(A):
        nn_, hh = np.nonzero(x[:, 1 + 6 * a, :, 0] < _PATCH_THR)
        boxes5[(nn_ * S + hh * W) * A + a, 0] = \
            32.0 * _sig(x[nn_, 1 + 6 * a, hh, 0])
        nn_, ww = np.nonzero(x[:, 2 + 6 * a, 0, :] < _PATCH_THR)
        boxes5[(nn_ * S + ww) * A + a, 1] = \
            32.0 * _sig(x[nn_, 2 + 6 * a, 0, ww])
    nn_, hh, ww = np.nonzero(x[:, 5] < _PATCH_THR)
    boxes5[(nn_ * S + hh * W + ww) * A, 2] = 60.0 * _sig(x[nn_, 5, hh, ww])

    logits = np.ascontiguousarray(
        x[:, 0::F, :, :].transpose(0, 2, 3, 1)
    ).reshape(-1)
    conf = _sig(logits)
    mask = conf >= np.float32(thr)
    k = int(mask.sum())
    sub = boxes5[mask]                                        # [k, 5]
    out = np.zeros((N * S * A, F), np.float32)
    out[:k, 0] = conf[mask]
    out[:k, 1] = sub[:, 0]
    out[:k, 2] = sub[:, 1]
    out[:k, 3] = sub[:, 3]
    out[:k, 4] = sub[:, 4]
    out[:k, 5] = sub[:, 2]
    return out


def _build_nc():
    """Build the per-core Bass module (same program on all 8 cores)."""
    import concourse.bacc as bacc
    import concourse.mybir as mybir
    import concourse.tile as tile

    f32 = mybir.dt.float32
    AF = mybir.ActivationFunctionType
    ALU = mybir.AluOpType

    nc = bacc.Bacc("TRN2", target_bir_lowering=False, debug=False)

    x = nc.dram_tensor("x", [M, C, H, W], f32, kind="ExternalInput")
    c1 = nc.dram_tensor("c1", [P, J], f32, kind="ExternalInput")
    c2 = nc.dram_tensor("c2", [P, J], f32, kind="ExternalInput")
    y = nc.dram_tensor("y", [M * S * A, F], f32, kind="ExternalOutput")

    # [M, C, S] view of the input; [M, P, 3600] view of the output where
    # partition p owns box rows [200p, 200p+200)*A of its image.
    xf = x.ap().rearrange("n c h w -> n c (h w)")
    yf = y.ap().rearrange("(n p q) f -> n p (q f)", n=M, p=P)

    ln_w = float(np.log(np.float32(ANCHOR_W)))
    ln_h = float(np.log(np.float32(ANCHOR_H)))

    with tile.TileContext(nc) as tc:
        with (
            tc.tile_pool(name="const", bufs=1) as constp,
            tc.tile_pool(name="inp", bufs=4) as inp,
            tc.tile_pool(name="outp", bufs=3) as outp,
            tc.tile_pool(name="tmp", bufs=2) as tmpp,
        ):
            c1_t = constp.tile([P, J], f32, tag="c1")
            nc.sync.dma_start(c1_t[:], c1.ap())
            c2_t = constp.tile([P, J], f32, tag="c2")
            nc.sync.dma_start(c2_t[:], c2.ap())
            bw_t = constp.tile([P, 1], f32, tag="bw")
            nc.vector.memset(bw_t[:], ln_w)
            bh_t = constp.tile([P, 1], f32, tag="bh")
            nc.vector.memset(bh_t[:], ln_h)
            # broadcast the [P, J] constants across the anchor dim
            c1v = c1_t[:].unsqueeze(1).broadcast_to([P, A, J])
            c2v = c2_t[:].unsqueeze(1).broadcast_to([P, A, J])

            def decode(inv, outv, outj, j0, j1):
                """Emit the 6 per-field pipelines for spatial cols [j0, j1)."""

                def tmp3(tag):
                    t = tmpp.tile([P, A * J], f32, tag=tag)
                    return t[:].rearrange("p (a j) -> p a j", a=A)[:, :, j0:j1]

                # f0: conf = 0.5 + 0.5*tanh(x/2)
                t0v = tmp3("t0")
                nc.scalar.activation(t0v, inv(0), AF.Tanh, scale=0.5)
                nc.vector.tensor_scalar(
                    out=outv(0), in0=t0v,
                    scalar1=0.5, scalar2=0.5, op0=ALU.mult, op1=ALU.add,
                )

                # f1: cx = (ix + sig)*32 = 16*(tanh + 2*ix + 1)
                t1v = tmp3("t1")
                nc.scalar.activation(t1v, inv(1), AF.Tanh, scale=0.5)
                u1v = tmp3("u1")
                nc.vector.tensor_add(u1v, t1v, c1v[:, :, j0:j1])
                nc.vector.tensor_scalar(
                    out=outv(1), in0=u1v, scalar1=16.0, scalar2=None,
                    op0=ALU.mult,
                )

                # f2: cy = 16*(tanh + 2*iy + 1)
                t2v = tmp3("t2")
                nc.scalar.activation(t2v, inv(2), AF.Tanh, scale=0.5)
                u2v = tmp3("u2")
                nc.vector.tensor_add(u2v, t2v, c2v[:, :, j0:j1])
                nc.vector.tensor_scalar(
                    out=outv(2), in0=u2v, scalar1=16.0, scalar2=None,
                    op0=ALU.mult,
                )

                # f3: w = exp(x + ln 85.72); f4: h = exp(x + ln 19.15)
                nc.scalar.activation(outv(3), inv(3), AF.Exp, bias=bw_t[:])
                nc.scalar.activation(outv(4), inv(4), AF.Exp, bias=bh_t[:])

                # f5: theta = (a + sig)*60 = 30*tanh + (60a + 30)
                t5v = tmp3("t5")
                nc.scalar.activation(t5v, inv(5), AF.Tanh, scale=0.5)
                for a in range(A):
                    nc.vector.tensor_scalar(
                        out=outj[:, F * a + 5, j0:j1],
                        in0=t5v[:, a],
                        scalar1=30.0, scalar2=60.0 * a + 30.0,
                        op0=ALU.mult, op1=ALU.add,
                    )

            for n in range(M):
                in_t = inp.tile([P, C * J], f32, tag="in")
                # channel c = a*6 + f sits at IN cols [c*J, (c+1)*J)
                invw = in_t[:].rearrange("p (a f j) -> p f a j", a=A, f=F)
                if n == 0:
                    # first image: per-field DMAs in pipeline order so the
                    # first ACT starts after 0.6MB instead of 1.84MB
                    for f in range(F):
                        nc.sync.dma_start(
                            invw[:, f],
                            xf[n].rearrange("(a f) (p j) -> f p a j",
                                            a=A, p=P)[f],
                        )
                else:
                    nc.sync.dma_start(
                        in_t[:].rearrange("p (c j) -> p c j", c=C),
                        xf[n].rearrange("c (p j) -> p c j", p=P),
                    )

                out_t = outp.tile([P, C * J], f32, tag="out")
                # OUT col = j*18 + a*6 + f  (row-major [76800, 6] boxes)
                outvw = out_t[:].rearrange("p (j a f) -> p f a j", a=A, f=F)
                outjw = out_t[:].rearrange("p (j c) -> p c j", c=C)

                halves = (0, J) if n < M - 1 else (0, J // 2, J)
                for h in range(len(halves) - 1):
                    j0, j1 = halves[h], halves[h + 1]
                    decode(lambda f: invw[:, f, :, j0:j1],
                           lambda f: outvw[:, f, :, j0:j1],
                           outjw, j0, j1)
                    # output rows for spatial cols [j0, j1) are contiguous
                    nc.sync.dma_start(
                        yf[n][:, j0 * C:j1 * C],
                        out_t[:, j0 * C:j1 * C],
                    )

    nc.compile()
    return nc


def _build_nc5():
    """Like _build_nc but the conf column is produced on the host (which
    already reads every conf logit for the compaction mask), so the device
    neither loads the 3 conf channels nor stores column 0: per-core traffic
    drops from 29.5MB to 24.6MB.

    Device output is the row-major [M*S*A, 5] matrix of (cx, cy, w, h, theta).
    """
    import concourse.bacc as bacc
    import concourse.mybir as mybir
    import concourse.tile as tile

    f32 = mybir.dt.float32
    AF = mybir.ActivationFunctionType
    ALU = mybir.AluOpType
    G = F - 1  # fields computed on device (1..5)

    nc = bacc.Bacc("TRN2", target_bir_lowering=False, debug=False)

    x = nc.dram_tensor("x", [M, C, H, W], f32, kind="ExternalInput")
    c1 = nc.dram_tensor("c1", [P, J], f32, kind="ExternalInput")
    c2 = nc.dram_tensor("c2", [P, J], f32, kind="ExternalInput")
    y = nc.dram_tensor("y", [M * S * A, G], f32, kind="ExternalOutput")

    xf = x.ap().rearrange("n c h w -> n c (h w)")
    yf = y.ap().rearrange("(n p q) f -> n p (q f)", n=M, p=P)

    ln_w = float(np.log(np.float32(ANCHOR_W)))
    ln_h = float(np.log(np.float32(ANCHOR_H)))

    with tile.TileContext(nc) as tc:
        with (
            tc.tile_pool(name="const", bufs=1) as constp,
            tc.tile_pool(name="inp", bufs=4) as inp,
            tc.tile_pool(name="outp", bufs=3) as outp,
            tc.tile_pool(name="tmp", bufs=2) as tmpp,
        ):
            c1_t = constp.tile([P, J], f32, tag="c1")
            nc.sync.dma_start(c1_t[:], c1.ap())
            c2_t = constp.tile([P, J], f32, tag="c2")
            nc.sync.dma_start(c2_t[:], c2.ap())
            bw_t = constp.tile([P, 1], f32, tag="bw")
            nc.vector.memset(bw_t[:], ln_w)
            bh_t = constp.tile([P, 1], f32, tag="bh")
            nc.vector.memset(bh_t[:], ln_h)
            c1v = c1_t[:].unsqueeze(1).broadcast_to([P, A, J])
            c2v = c2_t[:].unsqueeze(1).broadcast_to([P, A, J])

            def decode(inv, outv, outj, j0, j1):
                """fields 1..5 for spatial cols [j0, j1); conf is host-side."""

                def tmp3(tag):
                    t = tmpp.tile([P, A * J], f32, tag=tag)
                    return t[:].rearrange("p (a j) -> p a j", a=A)[:, :, j0:j1]

                # f1: cx = 16*(tanh + 2*ix + 1)
                t1v = tmp3("t1")
                nc.scalar.activation(t1v, inv(1), AF.Tanh, scale=0.5)
                u1v = tmp3("u1")
                nc.vector.tensor_add(u1v, t1v, c1v[:, :, j0:j1])
                nc.vector.tensor_scalar(
                    out=outv(1), in0=u1v, scalar1=16.0, scalar2=None,
                    op0=ALU.mult,
                )
                # f2: cy = 16*(tanh + 2*iy + 1)
                t2v = tmp3("t2")
                nc.scalar.activation(t2v, inv(2), AF.Tanh, scale=0.5)
                u2v = tmp3("u2")
                nc.vector.tensor_add(u2v, t2v, c2v[:, :, j0:j1])
                nc.vector.tensor_scalar(
                    out=outv(2), in0=u2v, scalar1=16.0, scalar2=None,
                    op0=ALU.mult,
                )
                # f3: w = exp(x + ln 85.72); f4: h = exp(x + ln 19.15)
                nc.scalar.activation(outv(3), inv(3), AF.Exp, bias=bw_t[:])
                nc.scalar.activation(outv(4), inv(4), AF.Exp, bias=bh_t[:])
                # f5: theta = 30*tanh + (60a + 30)
                t5v = tmp3("t5")
                nc.scalar.activation(t5v, inv(5), AF.Tanh, scale=0.5)
                for a in range(A):
                    nc.vector.tensor_scalar(
                        out=outj[:, G * a + 4, j0:j1],
                        in0=t5v[:, a],
                        scalar1=30.0, scalar2=60.0 * a + 30.0,
                        op0=ALU.mult, op1=ALU.add,
                    )

            C17 = C - 1  # channels 1..17 (conf channel 0 skipped; 6/12 dead)
            for n in range(M):
                # IN tile holds channels 1..17 in native order: channel c at
                # col (c-1)*J; field f anchor a -> c-1 = 6a + f - 1
                in_t = inp.tile([P, C17 * J], f32, tag="in")
                inw = in_t[:].rearrange("p (c j) -> p c j", c=C17)
                if n == 0:
                    # ramp: per-field DMAs in pipeline order
                    for f in range(1, F):
                        nc.sync.dma_start(
                            inw[:, f - 1:f + 12:F],
                            xf[n].rearrange("(a ff) (p j) -> ff p a j",
                                            a=A, p=P)[f],
                        )
                else:
                    # one DMA per image over the affine channel range 1..17
                    nc.sync.dma_start(
                        inw, xf[n][1:C].rearrange("c (p j) -> p c j", p=P),
                    )
                invw = None  # field views come from inw below

                out_t = outp.tile([P, A * G * J], f32, tag="out")
                # OUT col = j*15 + a*5 + (f-1)  (row-major [76800, 5])
                outvw = out_t[:].rearrange("p (j a f) -> p f a j", a=A, f=G)
                outjw = out_t[:].rearrange("p (j c) -> p c j", c=A * G)

                halves = (0, J) if n < M - 1 else (0, J // 2, J)
                for h in range(len(halves) - 1):
                    j0, j1 = halves[h], halves[h + 1]
                    decode(lambda f: inw[:, f - 1:f + 12:F, j0:j1],
                           lambda f: outvw[:, f - 1, :, j0:j1],
                           outjw, j0, j1)
                    nc.sync.dma_start(
                        yf[n][:, j0 * A * G:j1 * A * G],
                        out_t[:, j0 * A * G:j1 * A * G],
                    )

    nc.compile()
    return nc


def _build_nc_raw():
    """Hand-scheduled raw-bass variant: no TileContext barriers/preamble.

    Engine split: sync issues all input DMAs (HWDGE), scalar runs the 6 ACT
    ops per image, vector the 8 DVE ops, gpsimd issues output DMAs (SWDGE).
    Cyclic buffers (4x in, 3x out, 2x tmp) guarded by cumulative semaphore
    thresholds: s_in/s_out count DMA completions (x16), s_act/s_dve count
    compute ops.
    """
    from contextlib import ExitStack

    import concourse.bass as bass
    import concourse.mybir as mybir

    f32 = mybir.dt.float32
    AF = mybir.ActivationFunctionType
    ALU = mybir.AluOpType

    nc = bass.Bass("TRN2", target_bir_lowering=False, debug=False)

    x = nc.dram_tensor("x", [M, C, H, W], f32, kind="ExternalInput")
    # consts packed into one tensor: cols [0:J)=2*ix+1, [J:2J)=2*iy+1,
    # [2J]=ln(ANCHOR_W), [2J+1]=ln(ANCHOR_H)
    cc = nc.dram_tensor("cc", [P, 2 * J + 2], f32, kind="ExternalInput")
    y = nc.dram_tensor("y", [M * S * A, F], f32, kind="ExternalOutput")

    xf = x.ap().rearrange("n c h w -> n c (h w)")
    yf = y.ap().rearrange("(n p q) f -> n p (q f)", n=M, p=P)

    NBUF_IN, NBUF_OUT, NBUF_T = 5, 3, 2

    with ExitStack() as ctx:
        in_t = [ctx.enter_context(nc.sbuf_tensor(f"in{i}", [P, C * J], f32))
                for i in range(NBUF_IN)]
        out_t = [ctx.enter_context(nc.sbuf_tensor(f"out{i}", [P, C * J], f32))
                 for i in range(NBUF_OUT)]
        # tmp tanh tiles per field (t0,t1,t2,t5) and u tiles, double buffered
        tmps = {}
        for nm in ("t0", "t1", "t2", "t5", "u1", "u2"):
            tmps[nm] = [
                ctx.enter_context(nc.sbuf_tensor(f"{nm}_{i}", [P, A * J], f32))
                for i in range(NBUF_T)
            ]
        cc_t = ctx.enter_context(nc.sbuf_tensor("cc_t", [P, 2 * J + 2], f32))
        # one sem per DMA "slot" so milestone waits are never contaminated by
        # partial increments of a concurrently-running DMA on the same sem
        s_cc = ctx.enter_context(nc.semaphore("s_cc"))
        s_if = [ctx.enter_context(nc.semaphore(f"s_if{f}")) for f in range(F)]
        s_ib = [ctx.enter_context(nc.semaphore(f"s_ib{i}"))
                for i in range(NBUF_IN)]
        s_ih = [ctx.enter_context(nc.semaphore(f"s_ih{i}"))
                for i in range(NBUF_IN)]
        s_ob = [ctx.enter_context(nc.semaphore(f"s_ob{i}"))
                for i in range(NBUF_OUT)]
        s_act = ctx.enter_context(nc.semaphore("s_act"))
        s_dve = ctx.enter_context(nc.semaphore("s_dve"))
        block = ctx.enter_context(nc.Block())

        c1v = cc_t.ap()[:, 0:J].unsqueeze(1).broadcast_to([P, A, J])
        c2v = cc_t.ap()[:, J:2 * J].unsqueeze(1).broadcast_to([P, A, J])
        bw = cc_t.ap()[:, 2 * J:2 * J + 1]
        bh = cc_t.ap()[:, 2 * J + 1:2 * J + 2]

        # ---- static schedule bookkeeping (python-side counters) ----
        # input thresholds: img0 per-field on s_if[f]; img n>=1 split into a
        # low half (sync/HWDGE -> s_ib[n%4]) and high half (gpsimd/SWDGE ->
        # s_ih[n%4]); SWDGE and HWDGE must not share a semaphore
        def in_thrs(n):  # [(sem, value), ...] for image n loaded (n >= 1)
            v = 16 * ((n - 1) // NBUF_IN + 1)
            return [(s_ib[n % NBUF_IN], v)]

        # ACT op order: per image f0,f1,f2,f3,f4,f5 (img7: two j-halves)
        # DVE op order: f0ts, f1tt, f1ts, f2tt, f2ts, th0, th1, th2
        act_done_img = {}   # act count after image n's reads of in_t done
        dve_done_img = {}   # dve count after image n's writes to out_t done
        act_half = {}       # (n, h) -> act count after that half
        dve_half = {}
        # consumption points of tmp tiles (for ACT WAR on t*):
        dve_t_consumed = {}  # (name, n) -> dve count when t_name[n%2] free

        act_c = 0
        dve_c = 0
        for n in range(M):
            halves = (0, J) if n < M - 1 else (0, J // 2, J)
            for h in range(len(halves) - 1):
                act_c += 6
                dve_c += 8
                act_half[(n, h)] = act_c
                dve_half[(n, h)] = dve_c
            act_done_img[n] = act_c
            dve_done_img[n] = dve_c
            for nm in ("t0", "t1", "t2", "t5"):
                dve_t_consumed[(nm, n)] = dve_c  # conservative: end of image

        # per-out-buffer cumulative thresholds on s_ob[n%3]
        out_buf_cum = [0] * NBUF_OUT
        out_done_buf = {}   # n -> s_ob[n%3] value after image n's outs land
        for n in range(M):
            ndma = 2 if n == M - 1 else 1
            out_buf_cum[n % NBUF_OUT] += 16 * ndma
            out_done_buf[n] = out_buf_cum[n % NBUF_OUT]

        def img0_f_dma(eng, f):
            iv = in_t[0].ap().rearrange("p (a ff j) -> p ff a j",
                                        a=A, ff=F)[:, f]
            eng.dma_start(
                iv, xf[0].rearrange("(a ff) (p j) -> ff p a j",
                                    a=A, p=P)[f],
            ).then_inc(s_if[f], 16)

        # ---- sync engine: all input DMAs (one HWDGE ring) ----
        @block.sync
        def _(sync):
            for f in range(F):
                img0_f_dma(sync, f)
            for n in range(1, M):
                if n >= NBUF_IN:
                    sync.wait_ge(s_act, act_done_img[n - NBUF_IN])
                sync.dma_start(
                    in_t[n % NBUF_IN].ap().rearrange("p (c j) -> p c j", c=C),
                    xf[n].rearrange("c (p j) -> p c j", p=P),
                ).then_inc(s_ib[n % NBUF_IN], 16)

        # ---- scalar engine: ACT ops + high-half input DMAs ----
        @block.scalar
        def _(scalar):
            # dummy ACTIVATE before any wait so walrus's ACT_TABLE_LOAD for
            # exp_and_others runs during the input ramp, not after it
            const0 = nc.const_aps.aps[(f32, 0.0)]
            nc.scalar.activation(
                tmps["t0"][0].ap()[:, 0:1], const0[:, 0:1], AF.Tanh)
            scalar.dma_start(cc_t.ap(), cc.ap()).then_inc(s_cc, 16)
            scalar.wait_ge(s_cc, 16)  # exp bias tiles
            for n in range(M):
                ib = n % NBUF_IN
                ob = n % NBUF_OUT
                tb = n % NBUF_T
                invw = in_t[ib].ap().rearrange("p (a f j) -> p f a j",
                                               a=A, f=F)
                outvw = out_t[ob].ap().rearrange("p (j a f) -> p f a j",
                                                 a=A, f=F)
                halves = (0, J) if n < M - 1 else (0, J // 2, J)
                for h in range(len(halves) - 1):
                    j0, j1 = halves[h], halves[h + 1]
                    # data-ready wait
                    if n == 0:
                        pass  # per-f waits below
                    elif h == 0:
                        for sem, v in in_thrs(n):
                            scalar.wait_ge(sem, v)
                    # out_t WAR (f3/f4 write it)
                    if n >= NBUF_OUT and h == 0:
                        scalar.wait_ge(s_ob[n % NBUF_OUT],
                                       out_done_buf[n - NBUF_OUT])
                    # tmp WAR vs DVE of image n-2
                    if n >= NBUF_T and h == 0:
                        scalar.wait_ge(s_dve, dve_done_img[n - NBUF_T])

                    def tv(nm):
                        return tmps[nm][tb].ap().rearrange(
                            "p (a j) -> p a j", a=A)[:, :, j0:j1]

                    for f, func in ((0, AF.Tanh), (1, AF.Tanh), (2, AF.Tanh),
                                    (3, AF.Exp), (4, AF.Exp), (5, AF.Tanh)):
                        if n == 0:
                            scalar.wait_ge(s_if[f], 16)
                        iv = invw[:, f, :, j0:j1]
                        if func is AF.Exp:
                            b = bw if f == 3 else bh
                            inst = nc.scalar.activation(
                                outvw[:, f, :, j0:j1], iv, AF.Exp, bias=b)
                        else:
                            inst = nc.scalar.activation(
                                tv(f"t{f}" if f != 5 else "t5"), iv,
                                AF.Tanh, scale=0.5)
                        inst.then_inc(s_act, 1)

        # ---- vector engine: DVE ops ----
        @block.vector
        def _(vector):
            vector.wait_ge(s_cc, 16)  # consts loaded
            dve_c = 0
            u_read = {}  # (name, n) -> dve count after last read of u[name]
            for n in range(M):
                ob = n % NBUF_OUT
                tb = n % NBUF_T
                outvw = out_t[ob].ap().rearrange("p (j a f) -> p f a j",
                                                 a=A, f=F)
                outjw = out_t[ob].ap().rearrange("p (j c) -> p c j", c=C)
                halves = (0, J) if n < M - 1 else (0, J // 2, J)
                for h in range(len(halves) - 1):
                    j0, j1 = halves[h], halves[h + 1]
                    base_act = act_half[(n, h)] - 6

                    if n >= NBUF_OUT and h == 0:
                        vector.wait_ge(s_ob[n % NBUF_OUT],
                                       out_done_buf[n - NBUF_OUT])

                    def tv(nm):
                        return tmps[nm][tb].ap().rearrange(
                            "p (a j) -> p a j", a=A)[:, :, j0:j1]

                    # f0 conf
                    vector.wait_ge(s_act, base_act + 1)
                    nc.vector.tensor_scalar(
                        out=outvw[:, 0, :, j0:j1], in0=tv("t0"),
                        scalar1=0.5, scalar2=0.5,
                        op0=ALU.mult, op1=ALU.add,
                    ).then_inc(s_dve, 1)
                    dve_c += 1
                    # f1 cx (same-engine RAW on u1 and WAR vs image n-2)
                    vector.wait_ge(s_act, base_act + 2)
                    if ("u1", n - NBUF_T) in u_read:
                        vector.wait_ge(s_dve, u_read[("u1", n - NBUF_T)])
                    nc.vector.tensor_add(
                        tv("u1"), tv("t1"), c1v[:, :, j0:j1],
                    ).then_inc(s_dve, 1)
                    dve_c += 1
                    vector.wait_ge(s_dve, dve_c)
                    nc.vector.tensor_scalar(
                        out=outvw[:, 1, :, j0:j1], in0=tv("u1"),
                        scalar1=16.0, scalar2=None, op0=ALU.mult,
                    ).then_inc(s_dve, 1)
                    dve_c += 1
                    u_read[("u1", n)] = dve_c
                    # f2 cy
                    vector.wait_ge(s_act, base_act + 3)
                    if ("u2", n - NBUF_T) in u_read:
                        vector.wait_ge(s_dve, u_read[("u2", n - NBUF_T)])
                    nc.vector.tensor_add(
                        tv("u2"), tv("t2"), c2v[:, :, j0:j1],
                    ).then_inc(s_dve, 1)
                    dve_c += 1
                    vector.wait_ge(s_dve, dve_c)
                    nc.vector.tensor_scalar(
                        out=outvw[:, 2, :, j0:j1], in0=tv("u2"),
                        scalar1=16.0, scalar2=None, op0=ALU.mult,
                    ).then_inc(s_dve, 1)
                    dve_c += 1
                    u_read[("u2", n)] = dve_c
                    # f5 theta
                    vector.wait_ge(s_act, base_act + 6)
                    for a in range(A):
                        nc.vector.tensor_scalar(
                            out=outjw[:, F * a + 5, j0:j1],
                            in0=tv("t5")[:, a],
                            scalar1=30.0, scalar2=60.0 * a + 30.0,
                            op0=ALU.mult, op1=ALU.add,
                        ).then_inc(s_dve, 1)
                        dve_c += 1

        # ---- gpsimd engine (SWDGE): output DMAs ----
        @block.gpsimd
        def _(gpsimd):
            for n in range(M):
                ob = n % NBUF_OUT
                halves = (0, J) if n < M - 1 else (0, J // 2, J)
                for h in range(len(halves) - 1):
                    j0, j1 = halves[h], halves[h + 1]
                    gpsimd.wait_ge(s_act, act_half[(n, h)])
                    gpsimd.wait_ge(s_dve, dve_half[(n, h)])
                    gpsimd.dma_start(
                        yf[n][:, j0 * C:j1 * C],
                        out_t[ob].ap()[:, j0 * C:j1 * C],
                    ).then_inc(s_ob[ob], 16)
            for b in range(NBUF_OUT):
                gpsimd.wait_ge(s_ob[b], out_buf_cum[b])

    return nc


def _const_tiles():
    s = np.arange(S, dtype=np.int64).reshape(P, J)
    ix = (s % W).astype(np.float32)
    iy = (s // W).astype(np.float32)
    c1 = (2.0 * ix + 1.0).astype(np.float32)
    c2 = (2.0 * iy + 1.0).astype(np.float32)
    return np.ascontiguousarray(c1), np.ascontiguousarray(c2)


def _const_packed():
    c1, c2 = _const_tiles()
    ln_w = np.log(np.float32(ANCHOR_W)).astype(np.float32)
    ln_h = np.log(np.float32(ANCHOR_H)).astype(np.float32)
    tail = np.empty((P, 2), np.float32)
    tail[:, 0] = ln_w
    tail[:, 1] = ln_h
    return np.ascontiguousarray(np.concatenate([c1, c2, tail], axis=1))


def run(output, confidence_threshold, trace=False):
    """Run the kernel; returns (full_output, BassKernelResults)."""
    from concourse.bass_utils import run_bass_kernel_spmd

    x = np.asarray(output, dtype=np.float32)
    thr = float(np.asarray(confidence_threshold))
    assert x.shape == (N, C, H, W), x.shape

    import os
    impl = os.environ.get("DETECT_KERNEL_IMPL", "raw16b")
    builders = {"f16": _build_nc16, "raw16": _build_nc_raw16,
                "raw16b": _build_nc_raw16b,
                "tile5": _build_nc5, "tile": _build_nc, "raw": _build_nc_raw}
    if impl not in _nc_cache:
        _nc_cache[impl] = builders[impl]()
    nc = _nc_cache[impl]

    if impl in ("f16", "raw16", "raw16b"):
        xp = _pack_inputs16(x)
        in_maps = [{"x": xp[d]} for d in range(NCORES)]
        if impl != "raw16b":
            cc = _const16()
            for m_ in in_maps:
                m_["cc"] = cc
        res = run_bass_kernel_spmd(nc, in_maps, core_ids=list(range(NCORES)),
                                   trace=trace)
        return _unpack16(x, thr, res.results), res

    if impl == "raw":
        cc = _const_packed()
        in_maps = [
            {"x": np.ascontiguousarray(x[d * M:(d + 1) * M]), "cc": cc}
            for d in range(NCORES)
        ]
    else:
        c1, c2 = _const_tiles()
        in_maps = [
            {"x": np.ascontiguousarray(x[d * M:(d + 1) * M]),
             "c1": c1, "c2": c2}
            for d in range(NCORES)
        ]
    res = run_bass_kernel_spmd(nc, in_maps, core_ids=list(range(NCORES)),
                               trace=trace)
    boxes = np.concatenate([r["y"] for r in res.results], axis=0)

    # Stable compaction on host: valid rows (sigmoid(conf_logit) >= thr) first,
    # in original order; zero rows after. Mask from the raw logits in f32.
    logits = np.ascontiguousarray(
        x[:, 0::F, :, :].transpose(0, 2, 3, 1)
    ).reshape(-1)  # row order (n, h, w, a)
    conf = np.float32(1.0) / (np.float32(1.0) + np.exp(-logits))
    mask = conf >= np.float32(thr)
    k = int(mask.sum())
    out = np.zeros((N * S * A, F), np.float32)
    if impl == "tile5":
        # device produced (cx, cy, w, h, theta); conf column comes from the
        # same host sigmoid used for the mask
        out[:k, 0] = conf[mask]
        out[:k, 1:] = boxes[mask]
    else:
        out[:k] = boxes[mask]
    return out, res


def kernel(output, confidence_threshold):
    out, _ = run(output, confidence_threshold, trace=False)
    return out



# revision 26
# speedup vs baseline: 1.0384x; 1.0384x over previous
"""Trainium2 Bass kernel for nn_DetectMultiImage (YOLO-style box decode + compaction).

Contract: kernel(output, confidence_threshold) takes the FULL [64,18,160,160] f32
feature map, returns the FULL [4915200, 6] f32 boxes tensor (valid detections
first in row order, zero rows after), matching the jax reference.

Strategy: pure data parallel over the batch axis — 8 images per NeuronCore.
The default impl ("raw16b", hand-scheduled raw bass) moves fp16 instead of f32
(12.3MB per core vs 24.6MB — the kernel is chip-HBM-bandwidth-bound, so bytes
are the whole game; the 2e-2 harness tolerance leaves fp16 plenty of margin):

- Host packs the 15 useful channels per image into [P=128, 3000] fp16 tiles:
  a 1800-col tanh group (f1,f2,f5 x 3 anchors) and a 1200-col exp group
  (f3,f4) with ln(anchor_w/h) pre-added so both exp fields run as ONE
  unbiased ACT op. All DMA descriptors are contiguous 6000B/partition runs.
- Per image the device runs 2 ACT ops (tanh->tmp, exp->out) and 3 DVE
  scalar_tensor_tensor ops (out_plane = tanh*k + grid_const). Sigmoid is
  0.5 + 0.5*tanh(x/2) (tanh and exp share the exp_and_others ACT table; no
  table switches). Grid constants are generated on device by Pool iota + DVE
  arithmetic during the input ramp (no const DMA).
- Engine split: sync=HWDGE input DMAs, scalar=ACT, vector=DVE, gpsimd=SWDGE
  output DMAs, cyclic buffers (in x5 / out x4 / tmp x2) with cumulative
  semaphore thresholds. Order-sensitive waits get one semaphore per DMA slot.
  The out plane layout is (cx, cy, th, w, h) x (anchor-major 600).
- Host does what it must do anyway for compaction (sigmoid over the conf
  logits + the stable valid-first mask), plus the (p, a, j) -> row-order
  permute of the device planes and an exact recompute of the ~2k rows where
  fp16 tanh near -1 cancels against the +1 grid/anchor offset (ix=0 / iy=0 /
  a=0 with logit < -3).

Measured on trn2 (8 cores, NTFF profile of core 0): 43.8-48.9us (median ~44.2)
vs the 85-95us f32 baseline; the data window runs with zero DMA idle at
318-376GB/s/core, i.e. at the chip HBM roofline (variance = cross-core HBM
contention). Exec = ~8.7us fixed program preamble (instruction prefetch,
barriers, queue-reg loads) + data window + ~1.8us tail (DMA sem prop + exit
barrier, with the SWDGE dge_drain overlapped into the last transfers).
"""

import numpy as np

# Problem shape (hardcoded per harness contract)
N, C, H, W = 64, 18, 160, 160
A = 3                     # anchors
F = 6                     # fields per anchor: conf, cx, cy, w, h, theta
NCORES = 8
M = N // NCORES           # images per core
S = H * W                 # 25600 spatial positions
P = 128                   # SBUF partitions
J = S // P                # 200 spatial positions per partition per channel
CELL = 32.0
ANCHOR_W = 85.72
ANCHOR_H = 19.15
THETA_MARGIN = 60.0       # 180 / A

_nc_cache = {}

_LN_W = float(np.log(np.float32(ANCHOR_W)))
_LN_H = float(np.log(np.float32(ANCHOR_H)))


def _build_nc16():
    """fp16 I/O variant: per-core traffic drops to 12.3MB (vs 24.6MB f32).

    Host packs the 15 useful channels per image into a [P, 3000] fp16 tile:
    cols [0:1800) = tanh group (f1,f2,f5 x 3 anchors, 200 spatial each),
    cols [1800:3000) = exp group (f3,f4 x 3 anchors) with ln(anchor) pre-added
    so both exp fields fuse into one unbiased ACT op. Every DMA descriptor is
    a contiguous 6000B/partition run (>=512B, full DMA bus efficiency).

    Per image: 2 ACT ops (tanh 1800, exp 1200 -> out direct), 3 DVE
    scalar_tensor_tensor ops (out = tanh*k + const plane). Out tile [P, 3000]
    fp16 planes (cx, cy, th, w, h), each 600 cols a-major; host permutes
    (a, j) -> (j, a) row order during the compaction gather and patches the
    ~2k rows where fp16 tanh cancellation near -1 loses precision
    (ix=0 / iy=0 / a=0 with very negative logits).

    Input DMAs ride the sync/HWDGE ring; output DMAs issue from the Pool
    engine (SWDGE) so a not-yet-ready output never blocks input prefetch.
    """
    import concourse.bacc as bacc
    import concourse.mybir as mybir
    import concourse.tile as tile

    f16 = mybir.dt.float16
    AF = mybir.ActivationFunctionType
    ALU = mybir.AluOpType

    TG = 9 * J            # 1800 tanh-group cols
    EG = 6 * J            # 1200 exp-group cols
    TC = TG + EG          # 3000

    nc = bacc.Bacc("TRN2", target_bir_lowering=False, debug=False)

    x = nc.dram_tensor("x", [M, P, TC], f16, kind="ExternalInput")
    cc = nc.dram_tensor("cc", [P, TG], f16, kind="ExternalInput")
    y = nc.dram_tensor("y", [M, P, TC], f16, kind="ExternalOutput")

    with tile.TileContext(nc) as tc:
        with (
            tc.tile_pool(name="const", bufs=1) as constp,
            tc.tile_pool(name="inp", bufs=4) as inp,
            tc.tile_pool(name="outp", bufs=3) as outp,
            tc.tile_pool(name="tmp", bufs=3) as tmpp,
        ):
            cc_t = constp.tile([P, TG], f16, tag="cc")
            nc.sync.dma_start(cc_t[:], cc.ap())

            SC = (16.0, 16.0, 30.0)
            for n in range(M):
                in_t = inp.tile([P, TC], f16, tag="in")
                if n == 0:
                    # ramp: land the tanh group first so ACT starts early
                    nc.sync.dma_start(in_t[:, 0:TG], x.ap()[n][:, 0:TG])
                    nc.sync.dma_start(in_t[:, TG:TC], x.ap()[n][:, TG:TC])
                else:
                    nc.sync.dma_start(in_t[:], x.ap()[n])

                t_t = tmpp.tile([P, TG], f16, tag="t")
                out_t = outp.tile([P, TC], f16, tag="out")
                nc.scalar.activation(t_t[:], in_t[:, 0:TG], AF.Tanh, scale=0.5)
                nc.scalar.activation(out_t[:, TG:TC], in_t[:, TG:TC], AF.Exp)
                # w/h planes ready first: ship them without waiting on DVE
                nc.gpsimd.dma_start(y.ap()[n][:, TG:TC], out_t[:, TG:TC])
                for k in range(3):
                    a0, a1 = k * A * J, (k + 1) * A * J
                    nc.vector.scalar_tensor_tensor(
                        out_t[:, a0:a1], t_t[:, a0:a1], SC[k],
                        cc_t[:, a0:a1], ALU.mult, ALU.add,
                    )
                nc.gpsimd.dma_start(y.ap()[n][:, 0:TG], out_t[:, 0:TG])

    nc.compile()
    return nc


def _build_nc_raw16():
    """Hand-scheduled raw-bass version of the fp16 kernel (same math/layout
    as _build_nc16) — drops the TileContext preamble barriers/queue-reg loads
    and per-queue exit drains, which cost ~10us of the 47.6us tile-mode exec.

    Engine split: sync issues input DMAs (HWDGE), scalar runs the 2 ACT ops
    per image + the const DMA on its own ring, vector the 3 stt ops, gpsimd
    (SWDGE) the output DMAs. Cyclic buffers in x4 / out x3 / tanh-tmp x2
    guarded by cumulative semaphore thresholds; DMA completions inc by 16.
    """
    from contextlib import ExitStack

    import concourse.bass as bass
    import concourse.mybir as mybir

    f16 = mybir.dt.float16
    f32 = mybir.dt.float32
    AF = mybir.ActivationFunctionType
    ALU = mybir.AluOpType

    TG = 9 * J
    EG = 6 * J
    TC = TG + EG

    nc = bass.Bass("TRN2", target_bir_lowering=False, debug=False)
    x = nc.dram_tensor("x", [M, P, TC], f16, kind="ExternalInput")
    cc = nc.dram_tensor("cc", [P, TG], f16, kind="ExternalInput")
    y = nc.dram_tensor("y", [M, P, TC], f16, kind="ExternalOutput")

    NBI, NBO, NBT = 4, 3, 2
    SC = (16.0, 16.0, 30.0)

    with ExitStack() as ctx:
        in_t = [ctx.enter_context(nc.sbuf_tensor(f"in{i}", [P, TC], f16))
                for i in range(NBI)]
        out_t = [ctx.enter_context(nc.sbuf_tensor(f"out{i}", [P, TC], f16))
                 for i in range(NBO)]
        t_t = [ctx.enter_context(nc.sbuf_tensor(f"t{i}", [P, TG], f16))
               for i in range(NBT)]
        cc_t = ctx.enter_context(nc.sbuf_tensor("cc_t", [P, TG], f16))
        s_cc = ctx.enter_context(nc.semaphore("s_cc"))
        s_ib = [ctx.enter_context(nc.semaphore(f"s_ib{i}")) for i in range(NBI)]
        s_ob = [ctx.enter_context(nc.semaphore(f"s_ob{i}")) for i in range(NBO)]
        s_act = ctx.enter_context(nc.semaphore("s_act"))
        s_dve = ctx.enter_context(nc.semaphore("s_dve"))
        block = ctx.enter_context(nc.Block())

        # cumulative semaphore thresholds (python-side bookkeeping)
        in_cum = [0] * NBI
        in_thr_t = {}   # s_ib[n%4] value once image n's tanh-group cols landed
        in_thr_e = {}   # ... once the full image landed
        for n in range(M):
            b = n % NBI
            if n == 0:   # split ramp DMA: tanh group first
                in_cum[b] += 16
                in_thr_t[n] = in_cum[b]
                in_cum[b] += 16
                in_thr_e[n] = in_cum[b]
            else:
                in_cum[b] += 16
                in_thr_t[n] = in_thr_e[n] = in_cum[b]
        ob_cum = [0] * NBO
        ob_before = {}  # s_ob[n%3] value before image n may write out buffer
        for n in range(M):
            b = n % NBO
            ob_before[n] = ob_cum[b]
            ndma = 3 if n == M - 1 else 2
            ob_cum[b] += 16 * ndma

        @block.sync
        def _(sync):
            for n in range(M):
                b = n % NBI
                if n >= NBI:  # WAR: ACT reads of image n-4 done
                    sync.wait_ge(s_act, 2 * (n - NBI + 1))
                if n == 0:
                    sync.dma_start(in_t[0].ap()[:, 0:TG],
                                   x.ap()[0][:, 0:TG]).then_inc(s_ib[0], 16)
                    sync.dma_start(in_t[0].ap()[:, TG:TC],
                                   x.ap()[0][:, TG:TC]).then_inc(s_ib[0], 16)
                else:
                    sync.dma_start(in_t[b].ap(),
                                   x.ap()[n]).then_inc(s_ib[b], 16)

        @block.scalar
        def _(scalar):
            # dummy ACTIVATE before any wait so the exp_and_others
            # ACT_TABLE_LOAD runs during the input ramp
            const0 = nc.const_aps.aps[(f32, 0.0)]
            nc.scalar.activation(t_t[0].ap()[:, 0:1], const0[:, 0:1], AF.Tanh)
            scalar.dma_start(cc_t.ap(), cc.ap()).then_inc(s_cc, 16)
            for n in range(M):
                b, ob, tb = n % NBI, n % NBO, n % NBT
                scalar.wait_ge(s_ib[b], in_thr_t[n])
                if n >= NBT:  # WAR on t tile vs DVE reads of image n-2
                    scalar.wait_ge(s_dve, 3 * (n - NBT + 1))
                nc.scalar.activation(
                    t_t[tb].ap(), in_t[b].ap()[:, 0:TG], AF.Tanh, scale=0.5,
                ).then_inc(s_act, 1)
                if n == 0:
                    scalar.wait_ge(s_ib[0], in_thr_e[0])
                if n >= NBO:  # WAR on out tile vs out DMAs of image n-3
                    scalar.wait_ge(s_ob[ob], ob_before[n])
                nc.scalar.activation(
                    out_t[ob].ap()[:, TG:TC], in_t[b].ap()[:, TG:TC], AF.Exp,
                ).then_inc(s_act, 1)

        @block.vector
        def _(vector):
            vector.wait_ge(s_cc, 16)
            for n in range(M):
                ob, tb = n % NBO, n % NBT
                vector.wait_ge(s_act, 2 * n + 1)  # tanh of image n done
                if n >= NBO:
                    vector.wait_ge(s_ob[ob], ob_before[n])
                for k in range(3):
                    a0, a1 = k * A * J, (k + 1) * A * J
                    nc.vector.scalar_tensor_tensor(
                        out_t[ob].ap()[:, a0:a1], t_t[tb].ap()[:, a0:a1],
                        SC[k], cc_t.ap()[:, a0:a1], ALU.mult, ALU.add,
                    ).then_inc(s_dve, 1)

        @block.gpsimd
        def _(gpsimd):
            for n in range(M):
                ob = n % NBO
                gpsimd.wait_ge(s_act, 2 * n + 2)  # exp done: ship w/h planes
                gpsimd.dma_start(y.ap()[n][:, TG:TC],
                                 out_t[ob].ap()[:, TG:TC]).then_inc(s_ob[ob], 16)
                if n == M - 1:  # shorter drain tail: split the last DVE planes
                    gpsimd.wait_ge(s_dve, 3 * n + 2)
                    gpsimd.dma_start(
                        y.ap()[n][:, 0:2 * A * J],
                        out_t[ob].ap()[:, 0:2 * A * J]).then_inc(s_ob[ob], 16)
                    gpsimd.wait_ge(s_dve, 3 * n + 3)
                    gpsimd.dma_start(
                        y.ap()[n][:, 2 * A * J:TG],
                        out_t[ob].ap()[:, 2 * A * J:TG]).then_inc(s_ob[ob], 16)
                else:
                    gpsimd.wait_ge(s_dve, 3 * (n + 1))
                    gpsimd.dma_start(
                        y.ap()[n][:, 0:TG],
                        out_t[ob].ap()[:, 0:TG]).then_inc(s_ob[ob], 16)
            for b2 in range(NBO):
                gpsimd.wait_ge(s_ob[b2], ob_cum[b2])

    return nc


def _build_nc_raw16b(nbi=5, nbo=4):
    """raw16 + deeper buffering (in x5, out x4) + device-generated grid
    constants: Pool/gpsimd computes c1/c2/cth via iota+mod during the input
    ramp (Pool is otherwise idle there), dropping the 460KB const DMA from
    the DMA-bound window. No host const tensor at all.
    """
    from contextlib import ExitStack

    import concourse.bass as bass
    import concourse.mybir as mybir

    f16 = mybir.dt.float16
    f32 = mybir.dt.float32
    i32 = mybir.dt.int32
    AF = mybir.ActivationFunctionType
    ALU = mybir.AluOpType

    TG = 9 * J
    EG = 6 * J
    TC = TG + EG

    nc = bass.Bass("TRN2", target_bir_lowering=False, debug=False)
    x = nc.dram_tensor("x", [M, P, TC], f16, kind="ExternalInput")
    y = nc.dram_tensor("y", [M, P, TC], f16, kind="ExternalOutput")

    NBI, NBO, NBT = nbi, nbo, 2
    SC = (16.0, 16.0, 30.0)

    with ExitStack() as ctx:
        in_t = [ctx.enter_context(nc.sbuf_tensor(f"in{i}", [P, TC], f16))
                for i in range(NBI)]
        out_t = [ctx.enter_context(nc.sbuf_tensor(f"out{i}", [P, TC], f16))
                 for i in range(NBO)]
        t_t = [ctx.enter_context(nc.sbuf_tensor(f"t{i}", [P, TG], f16))
               for i in range(NBT)]
        qs_t = ctx.enter_context(nc.sbuf_tensor("qs", [P, J], i32))
        qy_t = ctx.enter_context(nc.sbuf_tensor("qy", [P, J], f32))
        q0_t = ctx.enter_context(nc.sbuf_tensor("q0", [P, J], f32))
        c1_t = ctx.enter_context(nc.sbuf_tensor("c1", [P, J], f16))
        c2_t = ctx.enter_context(nc.sbuf_tensor("c2", [P, J], f16))
        ct_t = ctx.enter_context(nc.sbuf_tensor("ct", [P, A * J], f16))
        s_iot = ctx.enter_context(nc.semaphore("s_iot"))
        s_i0 = ctx.enter_context(nc.semaphore("s_i0"))
        s_ib = [ctx.enter_context(nc.semaphore(f"s_ib{i}")) for i in range(NBI)]
        s_ob = [ctx.enter_context(nc.semaphore(f"s_ob{i}")) for i in range(NBO)]
        s_act = ctx.enter_context(nc.semaphore("s_act"))
        s_dve = ctx.enter_context(nc.semaphore("s_dve"))
        # keep the full exit dge_drain: skipping it (no_gpsimd_drain=True)
        # saved ~1.5us but left the SWDGE unit undrained at NEFF exit, which
        # intermittently faulted the NEXT execution with
        # NRT_EXEC_UNIT_UNRECOVERABLE (observed ~1 in 10 runs). Instead the
        # final output-completion waits sit on the idle sync engine, so the
        # drain overlaps the in-flight tail transfers (same ~1.5us back,
        # without skipping the quiesce).
        block = ctx.enter_context(nc.Block())

        # image 0's split input lands on two sems (s_i0 for the tanh group,
        # s_ib[0] for the exp group) so neither wait can be satisfied by the
        # other DMA completing first
        in_cum = [0] * NBI
        in_thr = {}
        for n in range(M):
            b = n % NBI
            in_cum[b] += 16
            in_thr[n] = in_cum[b]
        ob_cum = [0] * NBO
        ob_before = {}
        for n in range(M):
            b = n % NBO
            ob_before[n] = ob_cum[b]
            ndma = 3 if n == M - 1 else 2
            ob_cum[b] += 16 * ndma

        # broadcast views for the cx/cy in1 operands (one [P,J] column tile
        # replicated over the anchor dim with stride 0)
        c1v = c1_t.ap().unsqueeze(1).broadcast_to([P, A, J])
        c2v = c2_t.ap().unsqueeze(1).broadcast_to([P, A, J])

        @block.sync
        def _(sync):
            for n in range(M):
                b = n % NBI
                if n >= NBI:
                    sync.wait_ge(s_act, 2 * (n - NBI + 1))
                if n == 0:
                    sync.dma_start(in_t[0].ap()[:, 0:TG],
                                   x.ap()[0][:, 0:TG]).then_inc(s_i0, 16)
                    sync.dma_start(in_t[0].ap()[:, TG:TC],
                                   x.ap()[0][:, TG:TC]).then_inc(s_ib[0], 16)
                else:
                    sync.dma_start(in_t[b].ap(),
                                   x.ap()[n]).then_inc(s_ib[b], 16)
            # final output-completion waits live here (sync idles at the end)
            # rather than on gpsimd, so gpsimd reaches the Block exit right
            # after issuing the last DMA and its dge_drain overlaps the
            # in-flight transfers; the exit barrier still can't pass until
            # these waits prove every output byte (and its sem) landed
            for b2 in range(NBO):
                sync.wait_ge(s_ob[b2], ob_cum[b2])

        @block.scalar
        def _(scalar):
            const0 = nc.const_aps.aps[(f32, 0.0)]
            nc.scalar.activation(t_t[0].ap()[:, 0:1], const0[:, 0:1], AF.Tanh)
            for n in range(M):
                b, ob, tb = n % NBI, n % NBO, n % NBT
                if n == 0:
                    scalar.wait_ge(s_i0, 16)
                else:
                    scalar.wait_ge(s_ib[b], in_thr[n])
                if n >= NBT:
                    scalar.wait_ge(s_dve, 3 * (n - NBT + 1))
                nc.scalar.activation(
                    t_t[tb].ap(), in_t[b].ap()[:, 0:TG], AF.Tanh, scale=0.5,
                ).then_inc(s_act, 1)
                if n == 0:
                    scalar.wait_ge(s_ib[0], in_thr[0])
                if n >= NBO:
                    scalar.wait_ge(s_ob[ob], ob_before[n])
                nc.scalar.activation(
                    out_t[ob].ap()[:, TG:TC], in_t[b].ap()[:, TG:TC], AF.Exp,
                ).then_inc(s_act, 1)

        @block.vector
        def _(vector):
            # grid constants from the Pool iota (s = 200p + j), add/mult only:
            # iy = round((s - 79.5)/160) via the 2^23 round-to-int trick,
            # c2 = 32*iy + 16, c1 = 32*ix + 16 = (32*s + 16) - 5120*iy,
            # cth = 60a + 30 via three memsets.
            # 1.5*2^23: force round-to-integer at the f32 SBUF write (ulp=1
            # there); each op carries a sem inc so no pass can fold the chain
            TWO23 = 12582912.0
            vector.wait_ge(s_iot, 1)
            nc.vector.tensor_scalar(out=qy_t.ap(), in0=qs_t.ap(),
                                    scalar1=-79.5, scalar2=1.0 / W,
                                    op0=ALU.add, op1=ALU.mult).then_inc(s_iot, 1)
            nc.vector.tensor_scalar(out=q0_t.ap(), in0=qy_t.ap(),
                                    scalar1=TWO23, scalar2=None,
                                    op0=ALU.add).then_inc(s_iot, 1)
            nc.vector.tensor_scalar(out=qy_t.ap(), in0=q0_t.ap(),
                                    scalar1=-TWO23, scalar2=None,
                                    op0=ALU.add).then_inc(s_iot, 1)
            nc.vector.tensor_scalar(out=c2_t.ap(), in0=qy_t.ap(),
                                    scalar1=32.0, scalar2=16.0,
                                    op0=ALU.mult, op1=ALU.add).then_inc(s_iot, 1)
            nc.vector.tensor_scalar(out=q0_t.ap(), in0=qs_t.ap(),
                                    scalar1=32.0, scalar2=16.0,
                                    op0=ALU.mult, op1=ALU.add).then_inc(s_iot, 1)
            nc.vector.scalar_tensor_tensor(
                c1_t.ap(), qy_t.ap(), -32.0 * W, q0_t.ap(),
                ALU.mult, ALU.add).then_inc(s_iot, 1)
            for k in range(A):
                nc.vector.memset(ct_t.ap()[:, k * J:(k + 1) * J], 60.0 * k + 30.0)
            for n in range(M):
                ob, tb = n % NBO, n % NBT
                vector.wait_ge(s_act, 2 * n + 1)
                if n >= NBO:
                    vector.wait_ge(s_ob[ob], ob_before[n])
                ov = out_t[ob].ap()
                tv = t_t[tb].ap()
                nc.vector.scalar_tensor_tensor(
                    ov[:, 0:A * J].rearrange("p (a j) -> p a j", a=A),
                    tv[:, 0:A * J].rearrange("p (a j) -> p a j", a=A),
                    SC[0], c1v, ALU.mult, ALU.add,
                ).then_inc(s_dve, 1)
                nc.vector.scalar_tensor_tensor(
                    ov[:, A * J:2 * A * J].rearrange("p (a j) -> p a j", a=A),
                    tv[:, A * J:2 * A * J].rearrange("p (a j) -> p a j", a=A),
                    SC[1], c2v, ALU.mult, ALU.add,
                ).then_inc(s_dve, 1)
                nc.vector.scalar_tensor_tensor(
                    ov[:, 2 * A * J:TG], tv[:, 2 * A * J:TG],
                    SC[2], ct_t.ap(), ALU.mult, ALU.add,
                ).then_inc(s_dve, 1)

        @block.gpsimd
        def _(gpsimd):
            nc.gpsimd.iota(qs_t.ap(), [[1, J]], base=0,
                           channel_multiplier=J).then_inc(s_iot, 1)
            for n in range(M):
                ob = n % NBO
                gpsimd.wait_ge(s_act, 2 * n + 2)
                gpsimd.dma_start(y.ap()[n][:, TG:TC],
                                 out_t[ob].ap()[:, TG:TC]).then_inc(s_ob[ob], 16)
                if n == M - 1:
                    gpsimd.wait_ge(s_dve, 3 * n + 2)
                    gpsimd.dma_start(
                        y.ap()[n][:, 0:2 * A * J],
                        out_t[ob].ap()[:, 0:2 * A * J]).then_inc(s_ob[ob], 16)
                    gpsimd.wait_ge(s_dve, 3 * n + 3)
                    gpsimd.dma_start(
                        y.ap()[n][:, 2 * A * J:TG],
                        out_t[ob].ap()[:, 2 * A * J:TG]).then_inc(s_ob[ob], 16)
                else:
                    gpsimd.wait_ge(s_dve, 3 * (n + 1))
                    gpsimd.dma_start(
                        y.ap()[n][:, 0:TG],
                        out_t[ob].ap()[:, 0:TG]).then_inc(s_ob[ob], 16)

    return nc


def _build_nc_raw16c(nbi=5, nbo=4):
    """raw16b + offset-uint8 output planes: cx/cy/theta(a>=1) live in one
    grid/anchor cell, so the device ships q = round(tanh*127.5 + 127.5) as
    uint8 (exact round-to-nearest on DVE, <=0.5 step -> <=0.2% rel once the
    host adds back the 32*ix / 32*iy / 60*a cell base it already knows).
    theta(a=0) and w/h stay fp16. Output drops 6.14 -> 4.4MB/core (total
    10.6MB, -14%). No device-side grid constants needed at all.
    """
    from contextlib import ExitStack

    import concourse.bass as bass
    import concourse.mybir as mybir

    f16 = mybir.dt.float16
    f32 = mybir.dt.float32
    u8 = mybir.dt.uint8
    AF = mybir.ActivationFunctionType
    ALU = mybir.AluOpType

    TG = 9 * J            # 1800 tanh-group cols
    EG = 6 * J            # 1200 exp-group cols
    TC = TG + EG
    F16C = J + EG         # f16 out tile: [0:J) th0, [J:J+EG) w|h
    U8C = 2 * A * J + 2 * J   # u8 out tile: cx 600 | cy 600 | th12 400

    nc = bass.Bass("TRN2", target_bir_lowering=False, debug=False)
    x = nc.dram_tensor("x", [M, P, TC], f16, kind="ExternalInput")
    yf = nc.dram_tensor("yf", [M, P, F16C], f16, kind="ExternalOutput")
    yq = nc.dram_tensor("yq", [M, P, U8C], u8, kind="ExternalOutput")

    NBI, NBO, NBT = nbi, nbo, 2

    with ExitStack() as ctx:
        in_t = [ctx.enter_context(nc.sbuf_tensor(f"in{i}", [P, TC], f16))
                for i in range(NBI)]
        of_t = [ctx.enter_context(nc.sbuf_tensor(f"of{i}", [P, F16C], f16))
                for i in range(NBO)]
        oq_t = [ctx.enter_context(nc.sbuf_tensor(f"oq{i}", [P, U8C], u8))
                for i in range(NBO)]
        t_t = [ctx.enter_context(nc.sbuf_tensor(f"t{i}", [P, TG], f16))
               for i in range(NBT)]
        s_i0 = ctx.enter_context(nc.semaphore("s_i0"))
        s_ib = [ctx.enter_context(nc.semaphore(f"s_ib{i}")) for i in range(NBI)]
        s_ob = [ctx.enter_context(nc.semaphore(f"s_ob{i}")) for i in range(NBO)]
        s_act = ctx.enter_context(nc.semaphore("s_act"))
        s_dve = ctx.enter_context(nc.semaphore("s_dve"))
        block = ctx.enter_context(nc.Block())

        in_cum = [0] * NBI
        in_thr = {}
        for n in range(M):
            b = n % NBI
            in_cum[b] += 16
            in_thr[n] = in_cum[b]
        ob_cum = [0] * NBO
        ob_before = {}
        for n in range(M):
            b = n % NBO
            ob_before[n] = ob_cum[b]
            ob_cum[b] += 32          # 2 out DMAs x 16 per image

        @block.sync
        def _(sync):
            for n in range(M):
                b = n % NBI
                if n >= NBI:
                    sync.wait_ge(s_act, 2 * (n - NBI + 1))
                if n == 0:
                    sync.dma_start(in_t[0].ap()[:, 0:TG],
                                   x.ap()[0][:, 0:TG]).then_inc(s_i0, 16)
                    sync.dma_start(in_t[0].ap()[:, TG:TC],
                                   x.ap()[0][:, TG:TC]).then_inc(s_ib[0], 16)
                else:
                    sync.dma_start(in_t[b].ap(),
                                   x.ap()[n]).then_inc(s_ib[b], 16)
            for b2 in range(NBO):
                sync.wait_ge(s_ob[b2], ob_cum[b2])

        @block.scalar
        def _(scalar):
            const0 = nc.const_aps.aps[(f32, 0.0)]
            nc.scalar.activation(t_t[0].ap()[:, 0:1], const0[:, 0:1], AF.Tanh)
            for n in range(M):
                b, ob, tb = n % NBI, n % NBO, n % NBT
                if n == 0:
                    scalar.wait_ge(s_i0, 16)
                else:
                    scalar.wait_ge(s_ib[b], in_thr[n])
                if n >= NBT:
                    scalar.wait_ge(s_dve, 4 * (n - NBT + 1))
                nc.scalar.activation(
                    t_t[tb].ap(), in_t[b].ap()[:, 0:TG], AF.Tanh, scale=0.5,
                ).then_inc(s_act, 1)
                if n == 0:
                    scalar.wait_ge(s_ib[0], in_thr[0])
                if n >= NBO:
                    scalar.wait_ge(s_ob[ob], ob_before[n])
                nc.scalar.activation(
                    of_t[ob].ap()[:, J:F16C], in_t[b].ap()[:, TG:TC], AF.Exp,
                ).then_inc(s_act, 1)

        @block.vector
        def _(vector):
            for n in range(M):
                ob, tb = n % NBO, n % NBT
                vector.wait_ge(s_act, 2 * n + 1)
                if n >= NBO:
                    vector.wait_ge(s_ob[ob], ob_before[n])
                tv = t_t[tb].ap()
                # th0 (fp16) first so the f16 out DMA can ship early
                nc.vector.tensor_scalar(
                    out=of_t[ob].ap()[:, 0:J], in0=tv[:, 2 * A * J:2 * A * J + J],
                    scalar1=30.0, scalar2=30.0,
                    op0=ALU.mult, op1=ALU.add).then_inc(s_dve, 1)
                # cx, cy, th(a1,a2) as offset-uint8: q = t*127.5 + 127.5
                nc.vector.tensor_scalar(
                    out=oq_t[ob].ap()[:, 0:A * J], in0=tv[:, 0:A * J],
                    scalar1=127.5, scalar2=127.5,
                    op0=ALU.mult, op1=ALU.add).then_inc(s_dve, 1)
                nc.vector.tensor_scalar(
                    out=oq_t[ob].ap()[:, A * J:2 * A * J],
                    in0=tv[:, A * J:2 * A * J],
                    scalar1=127.5, scalar2=127.5,
                    op0=ALU.mult, op1=ALU.add).then_inc(s_dve, 1)
                nc.vector.tensor_scalar(
                    out=oq_t[ob].ap()[:, 2 * A * J:U8C],
                    in0=tv[:, 2 * A * J + J:TG],
                    scalar1=127.5, scalar2=127.5,
                    op0=ALU.mult, op1=ALU.add).then_inc(s_dve, 1)

        @block.gpsimd
        def _(gpsimd):
            for n in range(M):
                ob = n % NBO
                gpsimd.wait_ge(s_act, 2 * n + 2)       # exp done
                gpsimd.wait_ge(s_dve, 4 * n + 1)       # th0 done
                gpsimd.dma_start(yf.ap()[n],
                                 of_t[ob].ap()).then_inc(s_ob[ob], 16)
                gpsimd.wait_ge(s_dve, 4 * (n + 1))     # u8 planes done
                gpsimd.dma_start(yq.ap()[n],
                                 oq_t[ob].ap()).then_inc(s_ob[ob], 16)

    return nc


def _unpack16c(x, thr, results):
    """raw16c device planes -> full [N*S*A, 6] f32 boxes."""
    y16 = np.stack([np.asarray(r["yf"]) for r in results])   # [8,M,P,1400] f16
    yq = np.stack([np.asarray(r["yq"]) for r in results])    # [8,M,P,1600] u8
    s = np.arange(S, dtype=np.int64).reshape(P, J)
    ixv = (s % W).astype(np.float32) * 32.0                  # [P, J]
    iyv = (s // W).astype(np.float32) * 32.0
    QS = np.float32(16.0 / 127.5)
    QT = np.float32(60.0 / 255.0)

    def rows(a4):  # [8,M,P,A,J] -> row-ordered flat [N*S*A]
        return np.ascontiguousarray(a4.transpose(0, 1, 2, 4, 3)).reshape(-1)

    boxes5 = np.empty((N * S * A, 5), np.float32)
    q = yq[..., 0:A * J].reshape(NCORES, M, P, A, J).astype(np.float32)
    boxes5[:, 0] = rows(q * QS + ixv[None, None, :, None, :])
    q = yq[..., A * J:2 * A * J].reshape(NCORES, M, P, A, J).astype(np.float32)
    boxes5[:, 1] = rows(q * QS + iyv[None, None, :, None, :])
    th = np.empty((NCORES, M, P, A, J), np.float32)
    th[:, :, :, 0] = y16[..., 0:J].astype(np.float32)
    q = yq[..., 2 * A * J:].reshape(NCORES, M, P, 2, J).astype(np.float32)
    th[:, :, :, 1] = q[:, :, :, 0] * QT + 60.0
    th[:, :, :, 2] = q[:, :, :, 1] * QT + 120.0
    boxes5[:, 2] = rows(th)
    wh = y16[..., J:].reshape(NCORES, M, P, 2, A, J).astype(np.float32)
    boxes5[:, 3] = rows(wh[:, :, :, 0])
    boxes5[:, 4] = rows(wh[:, :, :, 1])

    # patches: uint8 offset error (<=0.063 abs) matters only where the cell
    # base is 0 AND the true value is small: ix=0 / iy=0 cols with logit
    # < -1.8 for cx/cy; theta a=0 (fp16) keeps the < -3 tanh-cancel patch
    U8THR = np.float32(-1.8)
    for a in range(A):
        nn_, hh = np.nonzero(x[:, 1 + 6 * a, :, 0] < U8THR)
        boxes5[(nn_ * S + hh * W) * A + a, 0] = \
            32.0 * _sig(x[nn_, 1 + 6 * a, hh, 0])
        nn_, ww = np.nonzero(x[:, 2 + 6 * a, 0, :] < U8THR)
        boxes5[(nn_ * S + ww) * A + a, 1] = \
            32.0 * _sig(x[nn_, 2 + 6 * a, 0, ww])
    nn_, hh, ww = np.nonzero(x[:, 5] < _PATCH_THR)
    boxes5[(nn_ * S + hh * W + ww) * A, 2] = 60.0 * _sig(x[nn_, 5, hh, ww])

    logits = np.ascontiguousarray(
        x[:, 0::F, :, :].transpose(0, 2, 3, 1)
    ).reshape(-1)
    conf = _sig(logits)
    mask = conf >= np.float32(thr)
    k = int(mask.sum())
    sub = boxes5[mask]
    out = np.zeros((N * S * A, F), np.float32)
    out[:k, 0] = conf[mask]
    out[:k, 1] = sub[:, 0]
    out[:k, 2] = sub[:, 1]
    out[:k, 3] = sub[:, 3]
    out[:k, 4] = sub[:, 4]
    out[:k, 5] = sub[:, 2]
    return out


def _pack_inputs16(x):
    """[N,C,H,W] f32 -> [NCORES, M, P, 3000] fp16 device layout."""
    xs = x.reshape(NCORES, M, C, P, J)
    CH = [1, 7, 13, 2, 8, 14, 5, 11, 17, 3, 9, 15, 4, 10, 16]
    arr = xs[:, :, CH].transpose(0, 1, 3, 2, 4)      # [8, M, P, 15, J]
    bias = np.zeros((15, 1), np.float32)
    bias[9:12] = _LN_W
    bias[12:15] = _LN_H
    packed = (arr + bias).astype(np.float16)
    return np.ascontiguousarray(packed.reshape(NCORES, M, P, 15 * J))


def _const16():
    s = np.arange(S, dtype=np.int64).reshape(P, J)
    ix = (s % W).astype(np.float32)
    iy = (s // W).astype(np.float32)
    c1s = np.broadcast_to((32 * ix + 16)[:, None, :], (P, A, J))
    c2s = np.broadcast_to((32 * iy + 16)[:, None, :], (P, A, J))
    cth = np.broadcast_to(
        (60 * np.arange(A, dtype=np.float32) + 30)[None, :, None], (P, A, J)
    )
    cc = np.concatenate(
        [c1s.reshape(P, A * J), c2s.reshape(P, A * J), cth.reshape(P, A * J)],
        axis=1,
    ).astype(np.float16)
    return np.ascontiguousarray(cc)


_PATCH_THR = np.float32(-3.0)


def _sig(v):
    return np.float32(1.0) / (np.float32(1.0) + np.exp(-v))


def _unpack16(x, thr, results):
    """Device planes -> full [N*S*A, 6] f32 boxes with stable compaction."""
    yb = np.stack([np.asarray(r["y"]) for r in results])     # [8,M,P,3000] f16
    # [8, M, P, plane(5), a, j] -> row order (n, p, j, a) x field
    v = yb.reshape(NCORES, M, P, 5, A, J).transpose(0, 1, 2, 5, 4, 3)
    boxes5 = np.ascontiguousarray(v).reshape(N * S * A, 5).astype(np.float32)
    # plane order: 0=cx, 1=cy, 2=theta, 3=w, 4=h

    # patch rows where fp16 tanh near -1 cancels against the +1 grid/anchor
    # offset (ix=0 / iy=0 / a=0 with logit < -3): recompute exactly on host.
    for a in range(A):
        nn_, hh = np.nonzero(x[:, 1 + 6 * a, :, 0] < _PATCH_THR)
        boxes5[(nn_ * S + hh * W) * A + a, 0] = \
            32.0 * _sig(x[nn_, 1 + 6 * a, hh, 0])
        nn_, ww = np.nonzero(x[:, 2 + 6 * a, 0, :] < _PATCH_THR)
        boxes5[(nn_ * S + ww) * A + a, 1] = \
            32.0 * _sig(x[nn_, 2 + 6 * a, 0, ww])
    nn_, hh, ww = np.nonzero(x[:, 5] < _PATCH_THR)
    boxes5[(nn_ * S + hh * W + ww) * A, 2] = 60.0 * _sig(x[nn_, 5, hh, ww])

    logits = np.ascontiguousarray(
        x[:, 0::F, :, :].transpose(0, 2, 3, 1)
    ).reshape(-1)
    conf = _sig(logits)
    mask = conf >= np.float32(thr)
    k = int(mask.sum())
    sub = boxes5[mask]                                        # [k, 5]
    out = np.zeros((N * S * A, F), np.float32)
    out[:k, 0] = conf[mask]
    out[:k, 1] = sub[:, 0]
    out[:k, 2] = sub[:, 1]
    out[:k, 3] = sub[:, 3]
    out[:k, 4] = sub[:, 4]
    out[:k, 5] = sub[:, 2]
    return out


def _build_nc():
    """Build the per-core Bass module (same program on all 8 cores)."""
    import concourse.bacc as bacc
    import concourse.mybir as mybir
    import concourse.tile as tile

    f32 = mybir.dt.float32
    AF = mybir.ActivationFunctionType
    ALU = mybir.AluOpType

    nc = bacc.Bacc("TRN2", target_bir_lowering=False, debug=False)

    x = nc.dram_tensor("x", [M, C, H, W], f32, kind="ExternalInput")
    c1 = nc.dram_tensor("c1", [P, J], f32, kind="ExternalInput")
    c2 = nc.dram_tensor("c2", [P, J], f32, kind="ExternalInput")
    y = nc.dram_tensor("y", [M * S * A, F], f32, kind="ExternalOutput")

    # [M, C, S] view of the input; [M, P, 3600] view of the output where
    # partition p owns box rows [200p, 200p+200)*A of its image.
    xf = x.ap().rearrange("n c h w -> n c (h w)")
    yf = y.ap().rearrange("(n p q) f -> n p (q f)", n=M, p=P)

    ln_w = float(np.log(np.float32(ANCHOR_W)))
    ln_h = float(np.log(np.float32(ANCHOR_H)))

    with tile.TileContext(nc) as tc:
        with (
            tc.tile_pool(name="const", bufs=1) as constp,
            tc.tile_pool(name="inp", bufs=4) as inp,
            tc.tile_pool(name="outp", bufs=3) as outp,
            tc.tile_pool(name="tmp", bufs=2) as tmpp,
        ):
            c1_t = constp.tile([P, J], f32, tag="c1")
            nc.sync.dma_start(c1_t[:], c1.ap())
            c2_t = constp.tile([P, J], f32, tag="c2")
            nc.sync.dma_start(c2_t[:], c2.ap())
            bw_t = constp.tile([P, 1], f32, tag="bw")
            nc.vector.memset(bw_t[:], ln_w)
            bh_t = constp.tile([P, 1], f32, tag="bh")
            nc.vector.memset(bh_t[:], ln_h)
            # broadcast the [P, J] constants across the anchor dim
            c1v = c1_t[:].unsqueeze(1).broadcast_to([P, A, J])
            c2v = c2_t[:].unsqueeze(1).broadcast_to([P, A, J])

            def decode(inv, outv, outj, j0, j1):
                """Emit the 6 per-field pipelines for spatial cols [j0, j1)."""

                def tmp3(tag):
                    t = tmpp.tile([P, A * J], f32, tag=tag)
                    return t[:].rearrange("p (a j) -> p a j", a=A)[:, :, j0:j1]

                # f0: conf = 0.5 + 0.5*tanh(x/2)
                t0v = tmp3("t0")
                nc.scalar.activation(t0v, inv(0), AF.Tanh, scale=0.5)
                nc.vector.tensor_scalar(
                    out=outv(0), in0=t0v,
                    scalar1=0.5, scalar2=0.5, op0=ALU.mult, op1=ALU.add,
                )

                # f1: cx = (ix + sig)*32 = 16*(tanh + 2*ix + 1)
                t1v = tmp3("t1")
                nc.scalar.activation(t1v, inv(1), AF.Tanh, scale=0.5)
                u1v = tmp3("u1")
                nc.vector.tensor_add(u1v, t1v, c1v[:, :, j0:j1])
                nc.vector.tensor_scalar(
                    out=outv(1), in0=u1v, scalar1=16.0, scalar2=None,
                    op0=ALU.mult,
                )

                # f2: cy = 16*(tanh + 2*iy + 1)
                t2v = tmp3("t2")
                nc.scalar.activation(t2v, inv(2), AF.Tanh, scale=0.5)
                u2v = tmp3("u2")
                nc.vector.tensor_add(u2v, t2v, c2v[:, :, j0:j1])
                nc.vector.tensor_scalar(
                    out=outv(2), in0=u2v, scalar1=16.0, scalar2=None,
                    op0=ALU.mult,
                )

                # f3: w = exp(x + ln 85.72); f4: h = exp(x + ln 19.15)
                nc.scalar.activation(outv(3), inv(3), AF.Exp, bias=bw_t[:])
                nc.scalar.activation(outv(4), inv(4), AF.Exp, bias=bh_t[:])

                # f5: theta = (a + sig)*60 = 30*tanh + (60a + 30)
                t5v = tmp3("t5")
                nc.scalar.activation(t5v, inv(5), AF.Tanh, scale=0.5)
                for a in range(A):
                    nc.vector.tensor_scalar(
                        out=outj[:, F * a + 5, j0:j1],
                        in0=t5v[:, a],
                        scalar1=30.0, scalar2=60.0 * a + 30.0,
                        op0=ALU.mult, op1=ALU.add,
                    )

            for n in range(M):
                in_t = inp.tile([P, C * J], f32, tag="in")
                # channel c = a*6 + f sits at IN cols [c*J, (c+1)*J)
                invw = in_t[:].rearrange("p (a f j) -> p f a j", a=A, f=F)
                if n == 0:
                    # first image: per-field DMAs in pipeline order so the
                    # first ACT starts after 0.6MB instead of 1.84MB
                    for f in range(F):
                        nc.sync.dma_start(
                            invw[:, f],
                            xf[n].rearrange("(a f) (p j) -> f p a j",
                                            a=A, p=P)[f],
                        )
                else:
                    nc.sync.dma_start(
                        in_t[:].rearrange("p (c j) -> p c j", c=C),
                        xf[n].rearrange("c (p j) -> p c j", p=P),
                    )

                out_t = outp.tile([P, C * J], f32, tag="out")
                # OUT col = j*18 + a*6 + f  (row-major [76800, 6] boxes)
                outvw = out_t[:].rearrange("p (j a f) -> p f a j", a=A, f=F)
                outjw = out_t[:].rearrange("p (j c) -> p c j", c=C)

                halves = (0, J) if n < M - 1 else (0, J // 2, J)
                for h in range(len(halves) - 1):
                    j0, j1 = halves[h], halves[h + 1]
                    decode(lambda f: invw[:, f, :, j0:j1],
                           lambda f: outvw[:, f, :, j0:j1],
                           outjw, j0, j1)
                    # output rows for spatial cols [j0, j1) are contiguous
                    nc.sync.dma_start(
                        yf[n][:, j0 * C:j1 * C],
                        out_t[:, j0 * C:j1 * C],
                    )

    nc.compile()
    return nc


def _build_nc5():
    """Like _build_nc but the conf column is produced on the host (which
    already reads every conf logit for the compaction mask), so the device
    neither loads the 3 conf channels nor stores column 0: per-core traffic
    drops from 29.5MB to 24.6MB.

    Device output is the row-major [M*S*A, 5] matrix of (cx, cy, w, h, theta).
    """
    import concourse.bacc as bacc
    import concourse.mybir as mybir
    import concourse.tile as tile

    f32 = mybir.dt.float32
    AF = mybir.ActivationFunctionType
    ALU = mybir.AluOpType
    G = F - 1  # fields computed on device (1..5)

    nc = bacc.Bacc("TRN2", target_bir_lowering=False, debug=False)

    x = nc.dram_tensor("x", [M, C, H, W], f32, kind="ExternalInput")
    c1 = nc.dram_tensor("c1", [P, J], f32, kind="ExternalInput")
    c2 = nc.dram_tensor("c2", [P, J], f32, kind="ExternalInput")
    y = nc.dram_tensor("y", [M * S * A, G], f32, kind="ExternalOutput")

    xf = x.ap().rearrange("n c h w -> n c (h w)")
    yf = y.ap().rearrange("(n p q) f -> n p (q f)", n=M, p=P)

    ln_w = float(np.log(np.float32(ANCHOR_W)))
    ln_h = float(np.log(np.float32(ANCHOR_H)))

    with tile.TileContext(nc) as tc:
        with (
            tc.tile_pool(name="const", bufs=1) as constp,
            tc.tile_pool(name="inp", bufs=4) as inp,
            tc.tile_pool(name="outp", bufs=3) as outp,
            tc.tile_pool(name="tmp", bufs=2) as tmpp,
        ):
            c1_t = constp.tile([P, J], f32, tag="c1")
            nc.sync.dma_start(c1_t[:], c1.ap())
            c2_t = constp.tile([P, J], f32, tag="c2")
            nc.sync.dma_start(c2_t[:], c2.ap())
            bw_t = constp.tile([P, 1], f32, tag="bw")
            nc.vector.memset(bw_t[:], ln_w)
            bh_t = constp.tile([P, 1], f32, tag="bh")
            nc.vector.memset(bh_t[:], ln_h)
            c1v = c1_t[:].unsqueeze(1).broadcast_to([P, A, J])
            c2v = c2_t[:].unsqueeze(1).broadcast_to([P, A, J])

            def decode(inv, outv, outj, j0, j1):
                """fields 1..5 for spatial cols [j0, j1); conf is host-side."""

                def tmp3(tag):
                    t = tmpp.tile([P, A * J], f32, tag=tag)
                    return t[:].rearrange("p (a j) -> p a j", a=A)[:, :, j0:j1]

                # f1: cx = 16*(tanh + 2*ix + 1)
                t1v = tmp3("t1")
                nc.scalar.activation(t1v, inv(1), AF.Tanh, scale=0.5)
                u1v = tmp3("u1")
                nc.vector.tensor_add(u1v, t1v, c1v[:, :, j0:j1])
                nc.vector.tensor_scalar(
                    out=outv(1), in0=u1v, scalar1=16.0, scalar2=None,
                    op0=ALU.mult,
                )
                # f2: cy = 16*(tanh + 2*iy + 1)
                t2v = tmp3("t2")
                nc.scalar.activation(t2v, inv(2), AF.Tanh, scale=0.5)
                u2v = tmp3("u2")
                nc.vector.tensor_add(u2v, t2v, c2v[:, :, j0:j1])
                nc.vector.tensor_scalar(
                    out=outv(2), in0=u2v, scalar1=16.0, scalar2=None,
                    op0=ALU.mult,
                )
                # f3: w = exp(x + ln 85.72); f4: h = exp(x + ln 19.15)
                nc.scalar.activation(outv(3), inv(3), AF.Exp, bias=bw_t[:])
                nc.scalar.activation(outv(4), inv(4), AF.Exp, bias=bh_t[:])
                # f5: theta = 30*tanh + (60a + 30)
                t5v = tmp3("t5")
                nc.scalar.activation(t5v, inv(5), AF.Tanh, scale=0.5)
                for a in range(A):
                    nc.vector.tensor_scalar(
                        out=outj[:, G * a + 4, j0:j1],
                        in0=t5v[:, a],
                        scalar1=30.0, scalar2=60.0 * a + 30.0,
                        op0=ALU.mult, op1=ALU.add,
                    )

            C17 = C - 1  # channels 1..17 (conf channel 0 skipped; 6/12 dead)
            for n in range(M):
                # IN tile holds channels 1..17 in native order: channel c at
                # col (c-1)*J; field f anchor a -> c-1 = 6a + f - 1
                in_t = inp.tile([P, C17 * J], f32, tag="in")
                inw = in_t[:].rearrange("p (c j) -> p c j", c=C17)
                if n == 0:
                    # ramp: per-field DMAs in pipeline order
                    for f in range(1, F):
                        nc.sync.dma_start(
                            inw[:, f - 1:f + 12:F],
                            xf[n].rearrange("(a ff) (p j) -> ff p a j",
                                            a=A, p=P)[f],
                        )
                else:
                    # one DMA per image over the affine channel range 1..17
                    nc.sync.dma_start(
                        inw, xf[n][1:C].rearrange("c (p j) -> p c j", p=P),
                    )
                invw = None  # field views come from inw below

                out_t = outp.tile([P, A * G * J], f32, tag="out")
                # OUT col = j*15 + a*5 + (f-1)  (row-major [76800, 5])
                outvw = out_t[:].rearrange("p (j a f) -> p f a j", a=A, f=G)
                outjw = out_t[:].rearrange("p (j c) -> p c j", c=A * G)

                halves = (0, J) if n < M - 1 else (0, J // 2, J)
                for h in range(len(halves) - 1):
                    j0, j1 = halves[h], halves[h + 1]
                    decode(lambda f: inw[:, f - 1:f + 12:F, j0:j1],
                           lambda f: outvw[:, f - 1, :, j0:j1],
                           outjw, j0, j1)
                    nc.sync.dma_start(
                        yf[n][:, j0 * A * G:j1 * A * G],
                        out_t[:, j0 * A * G:j1 * A * G],
                    )

    nc.compile()
    return nc


def _build_nc_raw():
    """Hand-scheduled raw-bass variant: no TileContext barriers/preamble.

    Engine split: sync issues all input DMAs (HWDGE), scalar runs the 6 ACT
    ops per image, vector the 8 DVE ops, gpsimd issues output DMAs (SWDGE).
    Cyclic buffers (4x in, 3x out, 2x tmp) guarded by cumulative semaphore
    thresholds: s_in/s_out count DMA completions (x16), s_act/s_dve count
    compute ops.
    """
    from contextlib import ExitStack

    import concourse.bass as bass
    import concourse.mybir as mybir

    f32 = mybir.dt.float32
    AF = mybir.ActivationFunctionType
    ALU = mybir.AluOpType

    nc = bass.Bass("TRN2", target_bir_lowering=False, debug=False)

    x = nc.dram_tensor("x", [M, C, H, W], f32, kind="ExternalInput")
    # consts packed into one tensor: cols [0:J)=2*ix+1, [J:2J)=2*iy+1,
    # [2J]=ln(ANCHOR_W), [2J+1]=ln(ANCHOR_H)
    cc = nc.dram_tensor("cc", [P, 2 * J + 2], f32, kind="ExternalInput")
    y = nc.dram_tensor("y", [M * S * A, F], f32, kind="ExternalOutput")

    xf = x.ap().rearrange("n c h w -> n c (h w)")
    yf = y.ap().rearrange("(n p q) f -> n p (q f)", n=M, p=P)

    NBUF_IN, NBUF_OUT, NBUF_T = 5, 3, 2

    with ExitStack() as ctx:
        in_t = [ctx.enter_context(nc.sbuf_tensor(f"in{i}", [P, C * J], f32))
                for i in range(NBUF_IN)]
        out_t = [ctx.enter_context(nc.sbuf_tensor(f"out{i}", [P, C * J], f32))
                 for i in range(NBUF_OUT)]
        # tmp tanh tiles per field (t0,t1,t2,t5) and u tiles, double buffered
        tmps = {}
        for nm in ("t0", "t1", "t2", "t5", "u1", "u2"):
            tmps[nm] = [
                ctx.enter_context(nc.sbuf_tensor(f"{nm}_{i}", [P, A * J], f32))
                for i in range(NBUF_T)
            ]
        cc_t = ctx.enter_context(nc.sbuf_tensor("cc_t", [P, 2 * J + 2], f32))
        # one sem per DMA "slot" so milestone waits are never contaminated by
        # partial increments of a concurrently-running DMA on the same sem
        s_cc = ctx.enter_context(nc.semaphore("s_cc"))
        s_if = [ctx.enter_context(nc.semaphore(f"s_if{f}")) for f in range(F)]
        s_ib = [ctx.enter_context(nc.semaphore(f"s_ib{i}"))
                for i in range(NBUF_IN)]
        s_ih = [ctx.enter_context(nc.semaphore(f"s_ih{i}"))
                for i in range(NBUF_IN)]
        s_ob = [ctx.enter_context(nc.semaphore(f"s_ob{i}"))
                for i in range(NBUF_OUT)]
        s_act = ctx.enter_context(nc.semaphore("s_act"))
        s_dve = ctx.enter_context(nc.semaphore("s_dve"))
        block = ctx.enter_context(nc.Block())

        c1v = cc_t.ap()[:, 0:J].unsqueeze(1).broadcast_to([P, A, J])
        c2v = cc_t.ap()[:, J:2 * J].unsqueeze(1).broadcast_to([P, A, J])
        bw = cc_t.ap()[:, 2 * J:2 * J + 1]
        bh = cc_t.ap()[:, 2 * J + 1:2 * J + 2]

        # ---- static schedule bookkeeping (python-side counters) ----
        # input thresholds: img0 per-field on s_if[f]; img n>=1 split into a
        # low half (sync/HWDGE -> s_ib[n%4]) and high half (gpsimd/SWDGE ->
        # s_ih[n%4]); SWDGE and HWDGE must not share a semaphore
        def in_thrs(n):  # [(sem, value), ...] for image n loaded (n >= 1)
            v = 16 * ((n - 1) // NBUF_IN + 1)
            return [(s_ib[n % NBUF_IN], v)]

        # ACT op order: per image f0,f1,f2,f3,f4,f5 (img7: two j-halves)
        # DVE op order: f0ts, f1tt, f1ts, f2tt, f2ts, th0, th1, th2
        act_done_img = {}   # act count after image n's reads of in_t done
        dve_done_img = {}   # dve count after image n's writes to out_t done
        act_half = {}       # (n, h) -> act count after that half
        dve_half = {}
        # consumption points of tmp tiles (for ACT WAR on t*):
        dve_t_consumed = {}  # (name, n) -> dve count when t_name[n%2] free

        act_c = 0
        dve_c = 0
        for n in range(M):
            halves = (0, J) if n < M - 1 else (0, J // 2, J)
            for h in range(len(halves) - 1):
                act_c += 6
                dve_c += 8
                act_half[(n, h)] = act_c
                dve_half[(n, h)] = dve_c
            act_done_img[n] = act_c
            dve_done_img[n] = dve_c
            for nm in ("t0", "t1", "t2", "t5"):
                dve_t_consumed[(nm, n)] = dve_c  # conservative: end of image

        # per-out-buffer cumulative thresholds on s_ob[n%3]
        out_buf_cum = [0] * NBUF_OUT
        out_done_buf = {}   # n -> s_ob[n%3] value after image n's outs land
        for n in range(M):
            ndma = 2 if n == M - 1 else 1
            out_buf_cum[n % NBUF_OUT] += 16 * ndma
            out_done_buf[n] = out_buf_cum[n % NBUF_OUT]

        def img0_f_dma(eng, f):
            iv = in_t[0].ap().rearrange("p (a ff j) -> p ff a j",
                                        a=A, ff=F)[:, f]
            eng.dma_start(
                iv, xf[0].rearrange("(a ff) (p j) -> ff p a j",
                                    a=A, p=P)[f],
            ).then_inc(s_if[f], 16)

        # ---- sync engine: all input DMAs (one HWDGE ring) ----
        @block.sync
        def _(sync):
            for f in range(F):
                img0_f_dma(sync, f)
            for n in range(1, M):
                if n >= NBUF_IN:
                    sync.wait_ge(s_act, act_done_img[n - NBUF_IN])
                sync.dma_start(
                    in_t[n % NBUF_IN].ap().rearrange("p (c j) -> p c j", c=C),
                    xf[n].rearrange("c (p j) -> p c j", p=P),
                ).then_inc(s_ib[n % NBUF_IN], 16)

        # ---- scalar engine: ACT ops + high-half input DMAs ----
        @block.scalar
        def _(scalar):
            # dummy ACTIVATE before any wait so walrus's ACT_TABLE_LOAD for
            # exp_and_others runs during the input ramp, not after it
            const0 = nc.const_aps.aps[(f32, 0.0)]
            nc.scalar.activation(
                tmps["t0"][0].ap()[:, 0:1], const0[:, 0:1], AF.Tanh)
            scalar.dma_start(cc_t.ap(), cc.ap()).then_inc(s_cc, 16)
            scalar.wait_ge(s_cc, 16)  # exp bias tiles
            for n in range(M):
                ib = n % NBUF_IN
                ob = n % NBUF_OUT
                tb = n % NBUF_T
                invw = in_t[ib].ap().rearrange("p (a f j) -> p f a j",
                                               a=A, f=F)
                outvw = out_t[ob].ap().rearrange("p (j a f) -> p f a j",
                                                 a=A, f=F)
                halves = (0, J) if n < M - 1 else (0, J // 2, J)
                for h in range(len(halves) - 1):
                    j0, j1 = halves[h], halves[h + 1]
                    # data-ready wait
                    if n == 0:
                        pass  # per-f waits below
                    elif h == 0:
                        for sem, v in in_thrs(n):
                            scalar.wait_ge(sem, v)
                    # out_t WAR (f3/f4 write it)
                    if n >= NBUF_OUT and h == 0:
                        scalar.wait_ge(s_ob[n % NBUF_OUT],
                                       out_done_buf[n - NBUF_OUT])
                    # tmp WAR vs DVE of image n-2
                    if n >= NBUF_T and h == 0:
                        scalar.wait_ge(s_dve, dve_done_img[n - NBUF_T])

                    def tv(nm):
                        return tmps[nm][tb].ap().rearrange(
                            "p (a j) -> p a j", a=A)[:, :, j0:j1]

                    for f, func in ((0, AF.Tanh), (1, AF.Tanh), (2, AF.Tanh),
                                    (3, AF.Exp), (4, AF.Exp), (5, AF.Tanh)):
                        if n == 0:
                            scalar.wait_ge(s_if[f], 16)
                        iv = invw[:, f, :, j0:j1]
                        if func is AF.Exp:
                            b = bw if f == 3 else bh
                            inst = nc.scalar.activation(
                                outvw[:, f, :, j0:j1], iv, AF.Exp, bias=b)
                        else:
                            inst = nc.scalar.activation(
                                tv(f"t{f}" if f != 5 else "t5"), iv,
                                AF.Tanh, scale=0.5)
                        inst.then_inc(s_act, 1)

        # ---- vector engine: DVE ops ----
        @block.vector
        def _(vector):
            vector.wait_ge(s_cc, 16)  # consts loaded
            dve_c = 0
            u_read = {}  # (name, n) -> dve count after last read of u[name]
            for n in range(M):
                ob = n % NBUF_OUT
                tb = n % NBUF_T
                outvw = out_t[ob].ap().rearrange("p (j a f) -> p f a j",
                                                 a=A, f=F)
                outjw = out_t[ob].ap().rearrange("p (j c) -> p c j", c=C)
                halves = (0, J) if n < M - 1 else (0, J // 2, J)
                for h in range(len(halves) - 1):
                    j0, j1 = halves[h], halves[h + 1]
                    base_act = act_half[(n, h)] - 6

                    if n >= NBUF_OUT and h == 0:
                        vector.wait_ge(s_ob[n % NBUF_OUT],
                                       out_done_buf[n - NBUF_OUT])

                    def tv(nm):
                        return tmps[nm][tb].ap().rearrange(
                            "p (a j) -> p a j", a=A)[:, :, j0:j1]

                    # f0 conf
                    vector.wait_ge(s_act, base_act + 1)
                    nc.vector.tensor_scalar(
                        out=outvw[:, 0, :, j0:j1], in0=tv("t0"),
                        scalar1=0.5, scalar2=0.5,
                        op0=ALU.mult, op1=ALU.add,
                    ).then_inc(s_dve, 1)
                    dve_c += 1
                    # f1 cx (same-engine RAW on u1 and WAR vs image n-2)
                    vector.wait_ge(s_act, base_act + 2)
                    if ("u1", n - NBUF_T) in u_read:
                        vector.wait_ge(s_dve, u_read[("u1", n - NBUF_T)])
                    nc.vector.tensor_add(
                        tv("u1"), tv("t1"), c1v[:, :, j0:j1],
                    ).then_inc(s_dve, 1)
                    dve_c += 1
                    vector.wait_ge(s_dve, dve_c)
                    nc.vector.tensor_scalar(
                        out=outvw[:, 1, :, j0:j1], in0=tv("u1"),
                        scalar1=16.0, scalar2=None, op0=ALU.mult,
                    ).then_inc(s_dve, 1)
                    dve_c += 1
                    u_read[("u1", n)] = dve_c
                    # f2 cy
                    vector.wait_ge(s_act, base_act + 3)
                    if ("u2", n - NBUF_T) in u_read:
                        vector.wait_ge(s_dve, u_read[("u2", n - NBUF_T)])
                    nc.vector.tensor_add(
                        tv("u2"), tv("t2"), c2v[:, :, j0:j1],
                    ).then_inc(s_dve, 1)
                    dve_c += 1
                    vector.wait_ge(s_dve, dve_c)
                    nc.vector.tensor_scalar(
                        out=outvw[:, 2, :, j0:j1], in0=tv("u2"),
                        scalar1=16.0, scalar2=None, op0=ALU.mult,
                    ).then_inc(s_dve, 1)
                    dve_c += 1
                    u_read[("u2", n)] = dve_c
                    # f5 theta
                    vector.wait_ge(s_act, base_act + 6)
                    for a in range(A):
                        nc.vector.tensor_scalar(
                            out=outjw[:, F * a + 5, j0:j1],
                            in0=tv("t5")[:, a],
                            scalar1=30.0, scalar2=60.0 * a + 30.0,
                            op0=ALU.mult, op1=ALU.add,
                        ).then_inc(s_dve, 1)
                        dve_c += 1

        # ---- gpsimd engine (SWDGE): output DMAs ----
        @block.gpsimd
        def _(gpsimd):
            for n in range(M):
                ob = n % NBUF_OUT
                halves = (0, J) if n < M - 1 else (0, J // 2, J)
                for h in range(len(halves) - 1):
                    j0, j1 = halves[h], halves[h + 1]
                    gpsimd.wait_ge(s_act, act_half[(n, h)])
                    gpsimd.wait_ge(s_dve, dve_half[(n, h)])
                    gpsimd.dma_start(
                        yf[n][:, j0 * C:j1 * C],
                        out_t[ob].ap()[:, j0 * C:j1 * C],
                    ).then_inc(s_ob[ob], 16)
            for b in range(NBUF_OUT):
                gpsimd.wait_ge(s_ob[b], out_buf_cum[b])

    return nc


def _const_tiles():
    s = np.arange(S, dtype=np.int64).reshape(P, J)
    ix = (s % W).astype(np.float32)
    iy = (s // W).astype(np.float32)
    c1 = (2.0 * ix + 1.0).astype(np.float32)
    c2 = (2.0 * iy + 1.0).astype(np.float32)
    return np.ascontiguousarray(c1), np.ascontiguousarray(c2)


def _const_packed():
    c1, c2 = _const_tiles()
    ln_w = np.log(np.float32(ANCHOR_W)).astype(np.float32)
    ln_h = np.log(np.float32(ANCHOR_H)).astype(np.float32)
    tail = np.empty((P, 2), np.float32)
    tail[:, 0] = ln_w
    tail[:, 1] = ln_h
    return np.ascontiguousarray(np.concatenate([c1, c2, tail], axis=1))


def run(output, confidence_threshold, trace=False):
    """Run the kernel; returns (full_output, BassKernelResults)."""
    from concourse.bass_utils import run_bass_kernel_spmd

    x = np.asarray(output, dtype=np.float32)
    thr = float(np.asarray(confidence_threshold))
    assert x.shape == (N, C, H, W), x.shape

    import os
    impl = os.environ.get("DETECT_KERNEL_IMPL", "raw16b")
    builders = {"f16": _build_nc16, "raw16": _build_nc_raw16,
                "raw16b": _build_nc_raw16b, "raw16c": _build_nc_raw16c,
                "tile5": _build_nc5, "tile": _build_nc, "raw": _build_nc_raw}
    if impl not in _nc_cache:
        _nc_cache[impl] = builders[impl]()
    nc = _nc_cache[impl]

    if impl in ("f16", "raw16", "raw16b", "raw16c"):
        xp = _pack_inputs16(x)
        in_maps = [{"x": xp[d]} for d in range(NCORES)]
        if impl not in ("raw16b", "raw16c"):
            cc = _const16()
            for m_ in in_maps:
                m_["cc"] = cc
        res = run_bass_kernel_spmd(nc, in_maps, core_ids=list(range(NCORES)),
                                   trace=trace)
        if impl == "raw16c":
            return _unpack16c(x, thr, res.results), res
        return _unpack16(x, thr, res.results), res

    if impl == "raw":
        cc = _const_packed()
        in_maps = [
            {"x": np.ascontiguousarray(x[d * M:(d + 1) * M]), "cc": cc}
            for d in range(NCORES)
        ]
    else:
        c1, c2 = _const_tiles()
        in_maps = [
            {"x": np.ascontiguousarray(x[d * M:(d + 1) * M]),
             "c1": c1, "c2": c2}
            for d in range(NCORES)
        ]
    res = run_bass_kernel_spmd(nc, in_maps, core_ids=list(range(NCORES)),
                               trace=trace)
    boxes = np.concatenate([r["y"] for r in res.results], axis=0)

    # Stable compaction on host: valid rows (sigmoid(conf_logit) >= thr) first,
    # in original order; zero rows after. Mask from the raw logits in f32.
    logits = np.ascontiguousarray(
        x[:, 0::F, :, :].transpose(0, 2, 3, 1)
    ).reshape(-1)  # row order (n, h, w, a)
    conf = np.float32(1.0) / (np.float32(1.0) + np.exp(-logits))
    mask = conf >= np.float32(thr)
    k = int(mask.sum())
    out = np.zeros((N * S * A, F), np.float32)
    if impl == "tile5":
        # device produced (cx, cy, w, h, theta); conf column comes from the
        # same host sigmoid used for the mask
        out[:k, 0] = conf[mask]
        out[:k, 1:] = boxes[mask]
    else:
        out[:k] = boxes[mask]
    return out, res


def kernel(output, confidence_threshold):
    out, _ = run(output, confidence_threshold, trace=False)
    return out



# revision 27
# speedup vs baseline: 1.0664x; 1.0270x over previous
"""Trainium2 Bass kernel for nn_DetectMultiImage (YOLO-style box decode + compaction).

Contract: kernel(output, confidence_threshold) takes the FULL [64,18,160,160] f32
feature map, returns the FULL [4915200, 6] f32 boxes tensor (valid detections
first in row order, zero rows after), matching the jax reference.

Strategy: pure data parallel over the batch axis — 8 images per NeuronCore.
The default impl ("raw16b", hand-scheduled raw bass) moves fp16 instead of f32
(12.3MB per core vs 24.6MB — the kernel is chip-HBM-bandwidth-bound, so bytes
are the whole game; the 2e-2 harness tolerance leaves fp16 plenty of margin):

- Host packs the 15 useful channels per image into [P=128, 3000] fp16 tiles:
  a 1800-col tanh group (f1,f2,f5 x 3 anchors) and a 1200-col exp group
  (f3,f4) with ln(anchor_w/h) pre-added so both exp fields run as ONE
  unbiased ACT op. All DMA descriptors are contiguous 6000B/partition runs.
- Per image the device runs 2 ACT ops (tanh->tmp, exp->out) and 3 DVE
  scalar_tensor_tensor ops (out_plane = tanh*k + grid_const). Sigmoid is
  0.5 + 0.5*tanh(x/2) (tanh and exp share the exp_and_others ACT table; no
  table switches). Grid constants are generated on device by Pool iota + DVE
  arithmetic during the input ramp (no const DMA).
- Engine split: sync=HWDGE input DMAs, scalar=ACT, vector=DVE, gpsimd=SWDGE
  output DMAs, cyclic buffers (in x5 / out x4 / tmp x2) with cumulative
  semaphore thresholds. Order-sensitive waits get one semaphore per DMA slot.
  The out plane layout is (cx, cy, th, w, h) x (anchor-major 600).
- Host does what it must do anyway for compaction (sigmoid over the conf
  logits + the stable valid-first mask), plus the (p, a, j) -> row-order
  permute of the device planes and an exact recompute of the ~2k rows where
  fp16 tanh near -1 cancels against the +1 grid/anchor offset (ix=0 / iy=0 /
  a=0 with logit < -3).

Measured on trn2 (8 cores, NTFF profile of core 0): 43.8-48.9us (median ~44.2)
vs the 85-95us f32 baseline; the data window runs with zero DMA idle at
318-376GB/s/core, i.e. at the chip HBM roofline (variance = cross-core HBM
contention). Exec = ~8.7us fixed program preamble (instruction prefetch,
barriers, queue-reg loads) + data window + ~1.8us tail (DMA sem prop + exit
barrier, with the SWDGE dge_drain overlapped into the last transfers).
"""

import numpy as np

# Problem shape (hardcoded per harness contract)
N, C, H, W = 64, 18, 160, 160
A = 3                     # anchors
F = 6                     # fields per anchor: conf, cx, cy, w, h, theta
NCORES = 8
M = N // NCORES           # images per core
S = H * W                 # 25600 spatial positions
P = 128                   # SBUF partitions
J = S // P                # 200 spatial positions per partition per channel
CELL = 32.0
ANCHOR_W = 85.72
ANCHOR_H = 19.15
THETA_MARGIN = 60.0       # 180 / A

_nc_cache = {}

_LN_W = float(np.log(np.float32(ANCHOR_W)))
_LN_H = float(np.log(np.float32(ANCHOR_H)))


def _build_nc16():
    """fp16 I/O variant: per-core traffic drops to 12.3MB (vs 24.6MB f32).

    Host packs the 15 useful channels per image into a [P, 3000] fp16 tile:
    cols [0:1800) = tanh group (f1,f2,f5 x 3 anchors, 200 spatial each),
    cols [1800:3000) = exp group (f3,f4 x 3 anchors) with ln(anchor) pre-added
    so both exp fields fuse into one unbiased ACT op. Every DMA descriptor is
    a contiguous 6000B/partition run (>=512B, full DMA bus efficiency).

    Per image: 2 ACT ops (tanh 1800, exp 1200 -> out direct), 3 DVE
    scalar_tensor_tensor ops (out = tanh*k + const plane). Out tile [P, 3000]
    fp16 planes (cx, cy, th, w, h), each 600 cols a-major; host permutes
    (a, j) -> (j, a) row order during the compaction gather and patches the
    ~2k rows where fp16 tanh cancellation near -1 loses precision
    (ix=0 / iy=0 / a=0 with very negative logits).

    Input DMAs ride the sync/HWDGE ring; output DMAs issue from the Pool
    engine (SWDGE) so a not-yet-ready output never blocks input prefetch.
    """
    import concourse.bacc as bacc
    import concourse.mybir as mybir
    import concourse.tile as tile

    f16 = mybir.dt.float16
    AF = mybir.ActivationFunctionType
    ALU = mybir.AluOpType

    TG = 9 * J            # 1800 tanh-group cols
    EG = 6 * J            # 1200 exp-group cols
    TC = TG + EG          # 3000

    nc = bacc.Bacc("TRN2", target_bir_lowering=False, debug=False)

    x = nc.dram_tensor("x", [M, P, TC], f16, kind="ExternalInput")
    cc = nc.dram_tensor("cc", [P, TG], f16, kind="ExternalInput")
    y = nc.dram_tensor("y", [M, P, TC], f16, kind="ExternalOutput")

    with tile.TileContext(nc) as tc:
        with (
            tc.tile_pool(name="const", bufs=1) as constp,
            tc.tile_pool(name="inp", bufs=4) as inp,
            tc.tile_pool(name="outp", bufs=3) as outp,
            tc.tile_pool(name="tmp", bufs=3) as tmpp,
        ):
            cc_t = constp.tile([P, TG], f16, tag="cc")
            nc.sync.dma_start(cc_t[:], cc.ap())

            SC = (16.0, 16.0, 30.0)
            for n in range(M):
                in_t = inp.tile([P, TC], f16, tag="in")
                if n == 0:
                    # ramp: land the tanh group first so ACT starts early
                    nc.sync.dma_start(in_t[:, 0:TG], x.ap()[n][:, 0:TG])
                    nc.sync.dma_start(in_t[:, TG:TC], x.ap()[n][:, TG:TC])
                else:
                    nc.sync.dma_start(in_t[:], x.ap()[n])

                t_t = tmpp.tile([P, TG], f16, tag="t")
                out_t = outp.tile([P, TC], f16, tag="out")
                nc.scalar.activation(t_t[:], in_t[:, 0:TG], AF.Tanh, scale=0.5)
                nc.scalar.activation(out_t[:, TG:TC], in_t[:, TG:TC], AF.Exp)
                # w/h planes ready first: ship them without waiting on DVE
                nc.gpsimd.dma_start(y.ap()[n][:, TG:TC], out_t[:, TG:TC])
                for k in range(3):
                    a0, a1 = k * A * J, (k + 1) * A * J
                    nc.vector.scalar_tensor_tensor(
                        out_t[:, a0:a1], t_t[:, a0:a1], SC[k],
                        cc_t[:, a0:a1], ALU.mult, ALU.add,
                    )
                nc.gpsimd.dma_start(y.ap()[n][:, 0:TG], out_t[:, 0:TG])

    nc.compile()
    return nc


def _build_nc_raw16():
    """Hand-scheduled raw-bass version of the fp16 kernel (same math/layout
    as _build_nc16) — drops the TileContext preamble barriers/queue-reg loads
    and per-queue exit drains, which cost ~10us of the 47.6us tile-mode exec.

    Engine split: sync issues input DMAs (HWDGE), scalar runs the 2 ACT ops
    per image + the const DMA on its own ring, vector the 3 stt ops, gpsimd
    (SWDGE) the output DMAs. Cyclic buffers in x4 / out x3 / tanh-tmp x2
    guarded by cumulative semaphore thresholds; DMA completions inc by 16.
    """
    from contextlib import ExitStack

    import concourse.bass as bass
    import concourse.mybir as mybir

    f16 = mybir.dt.float16
    f32 = mybir.dt.float32
    AF = mybir.ActivationFunctionType
    ALU = mybir.AluOpType

    TG = 9 * J
    EG = 6 * J
    TC = TG + EG

    nc = bass.Bass("TRN2", target_bir_lowering=False, debug=False)
    x = nc.dram_tensor("x", [M, P, TC], f16, kind="ExternalInput")
    cc = nc.dram_tensor("cc", [P, TG], f16, kind="ExternalInput")
    y = nc.dram_tensor("y", [M, P, TC], f16, kind="ExternalOutput")

    NBI, NBO, NBT = 4, 3, 2
    SC = (16.0, 16.0, 30.0)

    with ExitStack() as ctx:
        in_t = [ctx.enter_context(nc.sbuf_tensor(f"in{i}", [P, TC], f16))
                for i in range(NBI)]
        out_t = [ctx.enter_context(nc.sbuf_tensor(f"out{i}", [P, TC], f16))
                 for i in range(NBO)]
        t_t = [ctx.enter_context(nc.sbuf_tensor(f"t{i}", [P, TG], f16))
               for i in range(NBT)]
        cc_t = ctx.enter_context(nc.sbuf_tensor("cc_t", [P, TG], f16))
        s_cc = ctx.enter_context(nc.semaphore("s_cc"))
        s_ib = [ctx.enter_context(nc.semaphore(f"s_ib{i}")) for i in range(NBI)]
        s_ob = [ctx.enter_context(nc.semaphore(f"s_ob{i}")) for i in range(NBO)]
        s_act = ctx.enter_context(nc.semaphore("s_act"))
        s_dve = ctx.enter_context(nc.semaphore("s_dve"))
        block = ctx.enter_context(nc.Block())

        # cumulative semaphore thresholds (python-side bookkeeping)
        in_cum = [0] * NBI
        in_thr_t = {}   # s_ib[n%4] value once image n's tanh-group cols landed
        in_thr_e = {}   # ... once the full image landed
        for n in range(M):
            b = n % NBI
            if n == 0:   # split ramp DMA: tanh group first
                in_cum[b] += 16
                in_thr_t[n] = in_cum[b]
                in_cum[b] += 16
                in_thr_e[n] = in_cum[b]
            else:
                in_cum[b] += 16
                in_thr_t[n] = in_thr_e[n] = in_cum[b]
        ob_cum = [0] * NBO
        ob_before = {}  # s_ob[n%3] value before image n may write out buffer
        for n in range(M):
            b = n % NBO
            ob_before[n] = ob_cum[b]
            ndma = 3 if n == M - 1 else 2
            ob_cum[b] += 16 * ndma

        @block.sync
        def _(sync):
            for n in range(M):
                b = n % NBI
                if n >= NBI:  # WAR: ACT reads of image n-4 done
                    sync.wait_ge(s_act, 2 * (n - NBI + 1))
                if n == 0:
                    sync.dma_start(in_t[0].ap()[:, 0:TG],
                                   x.ap()[0][:, 0:TG]).then_inc(s_ib[0], 16)
                    sync.dma_start(in_t[0].ap()[:, TG:TC],
                                   x.ap()[0][:, TG:TC]).then_inc(s_ib[0], 16)
                else:
                    sync.dma_start(in_t[b].ap(),
                                   x.ap()[n]).then_inc(s_ib[b], 16)

        @block.scalar
        def _(scalar):
            # dummy ACTIVATE before any wait so the exp_and_others
            # ACT_TABLE_LOAD runs during the input ramp
            const0 = nc.const_aps.aps[(f32, 0.0)]
            nc.scalar.activation(t_t[0].ap()[:, 0:1], const0[:, 0:1], AF.Tanh)
            scalar.dma_start(cc_t.ap(), cc.ap()).then_inc(s_cc, 16)
            for n in range(M):
                b, ob, tb = n % NBI, n % NBO, n % NBT
                scalar.wait_ge(s_ib[b], in_thr_t[n])
                if n >= NBT:  # WAR on t tile vs DVE reads of image n-2
                    scalar.wait_ge(s_dve, 3 * (n - NBT + 1))
                nc.scalar.activation(
                    t_t[tb].ap(), in_t[b].ap()[:, 0:TG], AF.Tanh, scale=0.5,
                ).then_inc(s_act, 1)
                if n == 0:
                    scalar.wait_ge(s_ib[0], in_thr_e[0])
                if n >= NBO:  # WAR on out tile vs out DMAs of image n-3
                    scalar.wait_ge(s_ob[ob], ob_before[n])
                nc.scalar.activation(
                    out_t[ob].ap()[:, TG:TC], in_t[b].ap()[:, TG:TC], AF.Exp,
                ).then_inc(s_act, 1)

        @block.vector
        def _(vector):
            vector.wait_ge(s_cc, 16)
            for n in range(M):
                ob, tb = n % NBO, n % NBT
                vector.wait_ge(s_act, 2 * n + 1)  # tanh of image n done
                if n >= NBO:
                    vector.wait_ge(s_ob[ob], ob_before[n])
                for k in range(3):
                    a0, a1 = k * A * J, (k + 1) * A * J
                    nc.vector.scalar_tensor_tensor(
                        out_t[ob].ap()[:, a0:a1], t_t[tb].ap()[:, a0:a1],
                        SC[k], cc_t.ap()[:, a0:a1], ALU.mult, ALU.add,
                    ).then_inc(s_dve, 1)

        @block.gpsimd
        def _(gpsimd):
            for n in range(M):
                ob = n % NBO
                gpsimd.wait_ge(s_act, 2 * n + 2)  # exp done: ship w/h planes
                gpsimd.dma_start(y.ap()[n][:, TG:TC],
                                 out_t[ob].ap()[:, TG:TC]).then_inc(s_ob[ob], 16)
                if n == M - 1:  # shorter drain tail: split the last DVE planes
                    gpsimd.wait_ge(s_dve, 3 * n + 2)
                    gpsimd.dma_start(
                        y.ap()[n][:, 0:2 * A * J],
                        out_t[ob].ap()[:, 0:2 * A * J]).then_inc(s_ob[ob], 16)
                    gpsimd.wait_ge(s_dve, 3 * n + 3)
                    gpsimd.dma_start(
                        y.ap()[n][:, 2 * A * J:TG],
                        out_t[ob].ap()[:, 2 * A * J:TG]).then_inc(s_ob[ob], 16)
                else:
                    gpsimd.wait_ge(s_dve, 3 * (n + 1))
                    gpsimd.dma_start(
                        y.ap()[n][:, 0:TG],
                        out_t[ob].ap()[:, 0:TG]).then_inc(s_ob[ob], 16)
            for b2 in range(NBO):
                gpsimd.wait_ge(s_ob[b2], ob_cum[b2])

    return nc


def _build_nc_raw16b(nbi=5, nbo=4):
    """raw16 + deeper buffering (in x5, out x4) + device-generated grid
    constants: Pool/gpsimd computes c1/c2/cth via iota+mod during the input
    ramp (Pool is otherwise idle there), dropping the 460KB const DMA from
    the DMA-bound window. No host const tensor at all.
    """
    from contextlib import ExitStack

    import concourse.bass as bass
    import concourse.mybir as mybir

    f16 = mybir.dt.float16
    f32 = mybir.dt.float32
    i32 = mybir.dt.int32
    AF = mybir.ActivationFunctionType
    ALU = mybir.AluOpType

    TG = 9 * J
    EG = 6 * J
    TC = TG + EG

    nc = bass.Bass("TRN2", target_bir_lowering=False, debug=False)
    x = nc.dram_tensor("x", [M, P, TC], f16, kind="ExternalInput")
    y = nc.dram_tensor("y", [M, P, TC], f16, kind="ExternalOutput")

    NBI, NBO, NBT = nbi, nbo, 2
    SC = (16.0, 16.0, 30.0)

    with ExitStack() as ctx:
        in_t = [ctx.enter_context(nc.sbuf_tensor(f"in{i}", [P, TC], f16))
                for i in range(NBI)]
        out_t = [ctx.enter_context(nc.sbuf_tensor(f"out{i}", [P, TC], f16))
                 for i in range(NBO)]
        t_t = [ctx.enter_context(nc.sbuf_tensor(f"t{i}", [P, TG], f16))
               for i in range(NBT)]
        qs_t = ctx.enter_context(nc.sbuf_tensor("qs", [P, J], i32))
        qy_t = ctx.enter_context(nc.sbuf_tensor("qy", [P, J], f32))
        q0_t = ctx.enter_context(nc.sbuf_tensor("q0", [P, J], f32))
        c1_t = ctx.enter_context(nc.sbuf_tensor("c1", [P, J], f16))
        c2_t = ctx.enter_context(nc.sbuf_tensor("c2", [P, J], f16))
        ct_t = ctx.enter_context(nc.sbuf_tensor("ct", [P, A * J], f16))
        s_iot = ctx.enter_context(nc.semaphore("s_iot"))
        s_i0 = ctx.enter_context(nc.semaphore("s_i0"))
        s_ib = [ctx.enter_context(nc.semaphore(f"s_ib{i}")) for i in range(NBI)]
        s_ob = [ctx.enter_context(nc.semaphore(f"s_ob{i}")) for i in range(NBO)]
        s_act = ctx.enter_context(nc.semaphore("s_act"))
        s_dve = ctx.enter_context(nc.semaphore("s_dve"))
        # keep the full exit dge_drain: skipping it (no_gpsimd_drain=True)
        # saved ~1.5us but left the SWDGE unit undrained at NEFF exit, which
        # intermittently faulted the NEXT execution with
        # NRT_EXEC_UNIT_UNRECOVERABLE (observed ~1 in 10 runs). Instead the
        # final output-completion waits sit on the idle sync engine, so the
        # drain overlaps the in-flight tail transfers (same ~1.5us back,
        # without skipping the quiesce).
        block = ctx.enter_context(nc.Block())

        # image 0's split input lands on two sems (s_i0 for the tanh group,
        # s_ib[0] for the exp group) so neither wait can be satisfied by the
        # other DMA completing first
        in_cum = [0] * NBI
        in_thr = {}
        for n in range(M):
            b = n % NBI
            in_cum[b] += 16
            in_thr[n] = in_cum[b]
        ob_cum = [0] * NBO
        ob_before = {}
        for n in range(M):
            b = n % NBO
            ob_before[n] = ob_cum[b]
            ndma = 3 if n == M - 1 else 2
            ob_cum[b] += 16 * ndma

        # broadcast views for the cx/cy in1 operands (one [P,J] column tile
        # replicated over the anchor dim with stride 0)
        c1v = c1_t.ap().unsqueeze(1).broadcast_to([P, A, J])
        c2v = c2_t.ap().unsqueeze(1).broadcast_to([P, A, J])

        @block.sync
        def _(sync):
            for n in range(M):
                b = n % NBI
                if n >= NBI:
                    sync.wait_ge(s_act, 2 * (n - NBI + 1))
                if n == 0:
                    sync.dma_start(in_t[0].ap()[:, 0:TG],
                                   x.ap()[0][:, 0:TG]).then_inc(s_i0, 16)
                    sync.dma_start(in_t[0].ap()[:, TG:TC],
                                   x.ap()[0][:, TG:TC]).then_inc(s_ib[0], 16)
                else:
                    sync.dma_start(in_t[b].ap(),
                                   x.ap()[n]).then_inc(s_ib[b], 16)
            # final output-completion waits live here (sync idles at the end)
            # rather than on gpsimd, so gpsimd reaches the Block exit right
            # after issuing the last DMA and its dge_drain overlaps the
            # in-flight transfers; the exit barrier still can't pass until
            # these waits prove every output byte (and its sem) landed
            for b2 in range(NBO):
                sync.wait_ge(s_ob[b2], ob_cum[b2])

        @block.scalar
        def _(scalar):
            const0 = nc.const_aps.aps[(f32, 0.0)]
            nc.scalar.activation(t_t[0].ap()[:, 0:1], const0[:, 0:1], AF.Tanh)
            for n in range(M):
                b, ob, tb = n % NBI, n % NBO, n % NBT
                if n == 0:
                    scalar.wait_ge(s_i0, 16)
                else:
                    scalar.wait_ge(s_ib[b], in_thr[n])
                if n >= NBT:
                    scalar.wait_ge(s_dve, 3 * (n - NBT + 1))
                nc.scalar.activation(
                    t_t[tb].ap(), in_t[b].ap()[:, 0:TG], AF.Tanh, scale=0.5,
                ).then_inc(s_act, 1)
                if n == 0:
                    scalar.wait_ge(s_ib[0], in_thr[0])
                if n >= NBO:
                    scalar.wait_ge(s_ob[ob], ob_before[n])
                nc.scalar.activation(
                    out_t[ob].ap()[:, TG:TC], in_t[b].ap()[:, TG:TC], AF.Exp,
                ).then_inc(s_act, 1)

        @block.vector
        def _(vector):
            # grid constants from the Pool iota (s = 200p + j), add/mult only:
            # iy = round((s - 79.5)/160) via the 2^23 round-to-int trick,
            # c2 = 32*iy + 16, c1 = 32*ix + 16 = (32*s + 16) - 5120*iy,
            # cth = 60a + 30 via three memsets.
            # 1.5*2^23: force round-to-integer at the f32 SBUF write (ulp=1
            # there); each op carries a sem inc so no pass can fold the chain
            TWO23 = 12582912.0
            vector.wait_ge(s_iot, 1)
            nc.vector.tensor_scalar(out=qy_t.ap(), in0=qs_t.ap(),
                                    scalar1=-79.5, scalar2=1.0 / W,
                                    op0=ALU.add, op1=ALU.mult).then_inc(s_iot, 1)
            nc.vector.tensor_scalar(out=q0_t.ap(), in0=qy_t.ap(),
                                    scalar1=TWO23, scalar2=None,
                                    op0=ALU.add).then_inc(s_iot, 1)
            nc.vector.tensor_scalar(out=qy_t.ap(), in0=q0_t.ap(),
                                    scalar1=-TWO23, scalar2=None,
                                    op0=ALU.add).then_inc(s_iot, 1)
            nc.vector.tensor_scalar(out=c2_t.ap(), in0=qy_t.ap(),
                                    scalar1=32.0, scalar2=16.0,
                                    op0=ALU.mult, op1=ALU.add).then_inc(s_iot, 1)
            nc.vector.tensor_scalar(out=q0_t.ap(), in0=qs_t.ap(),
                                    scalar1=32.0, scalar2=16.0,
                                    op0=ALU.mult, op1=ALU.add).then_inc(s_iot, 1)
            nc.vector.scalar_tensor_tensor(
                c1_t.ap(), qy_t.ap(), -32.0 * W, q0_t.ap(),
                ALU.mult, ALU.add).then_inc(s_iot, 1)
            for k in range(A):
                nc.vector.memset(ct_t.ap()[:, k * J:(k + 1) * J], 60.0 * k + 30.0)
            for n in range(M):
                ob, tb = n % NBO, n % NBT
                vector.wait_ge(s_act, 2 * n + 1)
                if n >= NBO:
                    vector.wait_ge(s_ob[ob], ob_before[n])
                ov = out_t[ob].ap()
                tv = t_t[tb].ap()
                nc.vector.scalar_tensor_tensor(
                    ov[:, 0:A * J].rearrange("p (a j) -> p a j", a=A),
                    tv[:, 0:A * J].rearrange("p (a j) -> p a j", a=A),
                    SC[0], c1v, ALU.mult, ALU.add,
                ).then_inc(s_dve, 1)
                nc.vector.scalar_tensor_tensor(
                    ov[:, A * J:2 * A * J].rearrange("p (a j) -> p a j", a=A),
                    tv[:, A * J:2 * A * J].rearrange("p (a j) -> p a j", a=A),
                    SC[1], c2v, ALU.mult, ALU.add,
                ).then_inc(s_dve, 1)
                nc.vector.scalar_tensor_tensor(
                    ov[:, 2 * A * J:TG], tv[:, 2 * A * J:TG],
                    SC[2], ct_t.ap(), ALU.mult, ALU.add,
                ).then_inc(s_dve, 1)

        @block.gpsimd
        def _(gpsimd):
            nc.gpsimd.iota(qs_t.ap(), [[1, J]], base=0,
                           channel_multiplier=J).then_inc(s_iot, 1)
            for n in range(M):
                ob = n % NBO
                gpsimd.wait_ge(s_act, 2 * n + 2)
                gpsimd.dma_start(y.ap()[n][:, TG:TC],
                                 out_t[ob].ap()[:, TG:TC]).then_inc(s_ob[ob], 16)
                if n == M - 1:
                    gpsimd.wait_ge(s_dve, 3 * n + 2)
                    gpsimd.dma_start(
                        y.ap()[n][:, 0:2 * A * J],
                        out_t[ob].ap()[:, 0:2 * A * J]).then_inc(s_ob[ob], 16)
                    gpsimd.wait_ge(s_dve, 3 * n + 3)
                    gpsimd.dma_start(
                        y.ap()[n][:, 2 * A * J:TG],
                        out_t[ob].ap()[:, 2 * A * J:TG]).then_inc(s_ob[ob], 16)
                else:
                    gpsimd.wait_ge(s_dve, 3 * (n + 1))
                    gpsimd.dma_start(
                        y.ap()[n][:, 0:TG],
                        out_t[ob].ap()[:, 0:TG]).then_inc(s_ob[ob], 16)

    return nc


def _build_nc_raw16c(nbi=5, nbo=4):
    """raw16b + offset-uint8 output planes: cx/cy/theta(a>=1) live in one
    grid/anchor cell, so the device ships q = round(tanh*127.5 + 127.5) as
    uint8 (exact round-to-nearest on DVE, <=0.5 step -> <=0.2% rel once the
    host adds back the 32*ix / 32*iy / 60*a cell base it already knows).
    theta(a=0) and w/h stay fp16. Output drops 6.14 -> 4.4MB/core (total
    10.6MB, -14%). No device-side grid constants needed at all.
    """
    from contextlib import ExitStack

    import concourse.bass as bass
    import concourse.mybir as mybir

    f16 = mybir.dt.float16
    f32 = mybir.dt.float32
    u8 = mybir.dt.uint8
    AF = mybir.ActivationFunctionType
    ALU = mybir.AluOpType

    TG = 9 * J            # 1800 tanh-group cols
    EG = 6 * J            # 1200 exp-group cols
    TC = TG + EG
    F16C = J + EG         # f16 out tile: [0:J) th0, [J:J+EG) w|h
    U8C = 2 * A * J + 2 * J   # u8 out tile: cx 600 | cy 600 | th12 400

    nc = bass.Bass("TRN2", target_bir_lowering=False, debug=False)
    x = nc.dram_tensor("x", [M, P, TC], f16, kind="ExternalInput")
    yf = nc.dram_tensor("yf", [M, P, F16C], f16, kind="ExternalOutput")
    yq = nc.dram_tensor("yq", [M, P, U8C], u8, kind="ExternalOutput")

    NBI, NBO, NBT = nbi, nbo, 2

    with ExitStack() as ctx:
        in_t = [ctx.enter_context(nc.sbuf_tensor(f"in{i}", [P, TC], f16))
                for i in range(NBI)]
        of_t = [ctx.enter_context(nc.sbuf_tensor(f"of{i}", [P, F16C], f16))
                for i in range(NBO)]
        oq_t = [ctx.enter_context(nc.sbuf_tensor(f"oq{i}", [P, U8C], u8))
                for i in range(NBO)]
        t_t = [ctx.enter_context(nc.sbuf_tensor(f"t{i}", [P, TG], f16))
               for i in range(NBT)]
        s_i0 = ctx.enter_context(nc.semaphore("s_i0"))
        s_ib = [ctx.enter_context(nc.semaphore(f"s_ib{i}")) for i in range(NBI)]
        s_ob = [ctx.enter_context(nc.semaphore(f"s_ob{i}")) for i in range(NBO)]
        s_act = ctx.enter_context(nc.semaphore("s_act"))
        s_dve = ctx.enter_context(nc.semaphore("s_dve"))
        block = ctx.enter_context(nc.Block())

        in_cum = [0] * NBI
        in_thr = {}
        for n in range(M):
            b = n % NBI
            in_cum[b] += 16
            in_thr[n] = in_cum[b]
        ob_cum = [0] * NBO
        ob_before = {}
        for n in range(M):
            b = n % NBO
            ob_before[n] = ob_cum[b]
            ob_cum[b] += 32          # 2 out DMAs x 16 per image

        @block.sync
        def _(sync):
            for n in range(M):
                b = n % NBI
                if n >= NBI:
                    sync.wait_ge(s_act, 2 * (n - NBI + 1))
                if n == 0:
                    sync.dma_start(in_t[0].ap()[:, 0:TG],
                                   x.ap()[0][:, 0:TG]).then_inc(s_i0, 16)
                    sync.dma_start(in_t[0].ap()[:, TG:TC],
                                   x.ap()[0][:, TG:TC]).then_inc(s_ib[0], 16)
                else:
                    sync.dma_start(in_t[b].ap(),
                                   x.ap()[n]).then_inc(s_ib[b], 16)
            for b2 in range(NBO):
                sync.wait_ge(s_ob[b2], ob_cum[b2])

        @block.scalar
        def _(scalar):
            const0 = nc.const_aps.aps[(f32, 0.0)]
            nc.scalar.activation(t_t[0].ap()[:, 0:1], const0[:, 0:1], AF.Tanh)
            for n in range(M):
                b, ob, tb = n % NBI, n % NBO, n % NBT
                if n == 0:
                    scalar.wait_ge(s_i0, 16)
                else:
                    scalar.wait_ge(s_ib[b], in_thr[n])
                if n >= NBT:
                    scalar.wait_ge(s_dve, 4 * (n - NBT + 1))
                nc.scalar.activation(
                    t_t[tb].ap(), in_t[b].ap()[:, 0:TG], AF.Tanh, scale=0.5,
                ).then_inc(s_act, 1)
                if n == 0:
                    scalar.wait_ge(s_ib[0], in_thr[0])
                if n >= NBO:
                    scalar.wait_ge(s_ob[ob], ob_before[n])
                nc.scalar.activation(
                    of_t[ob].ap()[:, J:F16C], in_t[b].ap()[:, TG:TC], AF.Exp,
                ).then_inc(s_act, 1)

        @block.vector
        def _(vector):
            for n in range(M):
                ob, tb = n % NBO, n % NBT
                vector.wait_ge(s_act, 2 * n + 1)
                if n >= NBO:
                    vector.wait_ge(s_ob[ob], ob_before[n])
                tv = t_t[tb].ap()
                # th0 (fp16) first so the f16 out DMA can ship early
                nc.vector.tensor_scalar(
                    out=of_t[ob].ap()[:, 0:J], in0=tv[:, 2 * A * J:2 * A * J + J],
                    scalar1=30.0, scalar2=30.0,
                    op0=ALU.mult, op1=ALU.add).then_inc(s_dve, 1)
                # cx, cy, th(a1,a2) as offset-uint8: q = t*127.5 + 127.5
                nc.vector.tensor_scalar(
                    out=oq_t[ob].ap()[:, 0:A * J], in0=tv[:, 0:A * J],
                    scalar1=127.5, scalar2=127.5,
                    op0=ALU.mult, op1=ALU.add).then_inc(s_dve, 1)
                nc.vector.tensor_scalar(
                    out=oq_t[ob].ap()[:, A * J:2 * A * J],
                    in0=tv[:, A * J:2 * A * J],
                    scalar1=127.5, scalar2=127.5,
                    op0=ALU.mult, op1=ALU.add).then_inc(s_dve, 1)
                nc.vector.tensor_scalar(
                    out=oq_t[ob].ap()[:, 2 * A * J:U8C],
                    in0=tv[:, 2 * A * J + J:TG],
                    scalar1=127.5, scalar2=127.5,
                    op0=ALU.mult, op1=ALU.add).then_inc(s_dve, 1)

        @block.gpsimd
        def _(gpsimd):
            for n in range(M):
                ob = n % NBO
                gpsimd.wait_ge(s_act, 2 * n + 2)       # exp done
                gpsimd.wait_ge(s_dve, 4 * n + 1)       # th0 done
                gpsimd.dma_start(yf.ap()[n],
                                 of_t[ob].ap()).then_inc(s_ob[ob], 16)
                gpsimd.wait_ge(s_dve, 4 * (n + 1))     # u8 planes done
                gpsimd.dma_start(yq.ap()[n],
                                 oq_t[ob].ap()).then_inc(s_ob[ob], 16)

    return nc


def _unpack16c(x, thr, results):
    """raw16c device planes -> full [N*S*A, 6] f32 boxes."""
    y16 = np.stack([np.asarray(r["yf"]) for r in results])   # [8,M,P,1400] f16
    yq = np.stack([np.asarray(r["yq"]) for r in results])    # [8,M,P,1600] u8
    s = np.arange(S, dtype=np.int64).reshape(P, J)
    ixv = (s % W).astype(np.float32) * 32.0                  # [P, J]
    iyv = (s // W).astype(np.float32) * 32.0
    QS = np.float32(16.0 / 127.5)
    QT = np.float32(60.0 / 255.0)

    def rows(a4):  # [8,M,P,A,J] -> row-ordered flat [N*S*A]
        return np.ascontiguousarray(a4.transpose(0, 1, 2, 4, 3)).reshape(-1)

    boxes5 = np.empty((N * S * A, 5), np.float32)
    q = yq[..., 0:A * J].reshape(NCORES, M, P, A, J).astype(np.float32)
    boxes5[:, 0] = rows(q * QS + ixv[None, None, :, None, :])
    q = yq[..., A * J:2 * A * J].reshape(NCORES, M, P, A, J).astype(np.float32)
    boxes5[:, 1] = rows(q * QS + iyv[None, None, :, None, :])
    th = np.empty((NCORES, M, P, A, J), np.float32)
    th[:, :, :, 0] = y16[..., 0:J].astype(np.float32)
    q = yq[..., 2 * A * J:].reshape(NCORES, M, P, 2, J).astype(np.float32)
    th[:, :, :, 1] = q[:, :, :, 0] * QT + 60.0
    th[:, :, :, 2] = q[:, :, :, 1] * QT + 120.0
    boxes5[:, 2] = rows(th)
    wh = y16[..., J:].reshape(NCORES, M, P, 2, A, J).astype(np.float32)
    boxes5[:, 3] = rows(wh[:, :, :, 0])
    boxes5[:, 4] = rows(wh[:, :, :, 1])

    # patches: uint8 offset error (<=0.063 abs) matters only where the cell
    # base is 0 AND the true value is small: ix=0 / iy=0 cols with logit
    # < -1.8 for cx/cy; theta a=0 (fp16) keeps the < -3 tanh-cancel patch
    U8THR = np.float32(-1.2)
    for a in range(A):
        nn_, hh = np.nonzero(x[:, 1 + 6 * a, :, 0] < U8THR)
        boxes5[(nn_ * S + hh * W) * A + a, 0] = \
            32.0 * _sig(x[nn_, 1 + 6 * a, hh, 0])
        nn_, ww = np.nonzero(x[:, 2 + 6 * a, 0, :] < U8THR)
        boxes5[(nn_ * S + ww) * A + a, 1] = \
            32.0 * _sig(x[nn_, 2 + 6 * a, 0, ww])
    nn_, hh, ww = np.nonzero(x[:, 5] < _PATCH_THR)
    boxes5[(nn_ * S + hh * W + ww) * A, 2] = 60.0 * _sig(x[nn_, 5, hh, ww])

    logits = np.ascontiguousarray(
        x[:, 0::F, :, :].transpose(0, 2, 3, 1)
    ).reshape(-1)
    conf = _sig(logits)
    mask = conf >= np.float32(thr)
    k = int(mask.sum())
    sub = boxes5[mask]
    out = np.zeros((N * S * A, F), np.float32)
    out[:k, 0] = conf[mask]
    out[:k, 1] = sub[:, 0]
    out[:k, 2] = sub[:, 1]
    out[:k, 3] = sub[:, 3]
    out[:k, 4] = sub[:, 4]
    out[:k, 5] = sub[:, 2]
    return out


def _pack_inputs16(x):
    """[N,C,H,W] f32 -> [NCORES, M, P, 3000] fp16 device layout."""
    xs = x.reshape(NCORES, M, C, P, J)
    CH = [1, 7, 13, 2, 8, 14, 5, 11, 17, 3, 9, 15, 4, 10, 16]
    arr = xs[:, :, CH].transpose(0, 1, 3, 2, 4)      # [8, M, P, 15, J]
    bias = np.zeros((15, 1), np.float32)
    bias[9:12] = _LN_W
    bias[12:15] = _LN_H
    packed = (arr + bias).astype(np.float16)
    return np.ascontiguousarray(packed.reshape(NCORES, M, P, 15 * J))


def _const16():
    s = np.arange(S, dtype=np.int64).reshape(P, J)
    ix = (s % W).astype(np.float32)
    iy = (s // W).astype(np.float32)
    c1s = np.broadcast_to((32 * ix + 16)[:, None, :], (P, A, J))
    c2s = np.broadcast_to((32 * iy + 16)[:, None, :], (P, A, J))
    cth = np.broadcast_to(
        (60 * np.arange(A, dtype=np.float32) + 30)[None, :, None], (P, A, J)
    )
    cc = np.concatenate(
        [c1s.reshape(P, A * J), c2s.reshape(P, A * J), cth.reshape(P, A * J)],
        axis=1,
    ).astype(np.float16)
    return np.ascontiguousarray(cc)


_PATCH_THR = np.float32(-3.0)


def _sig(v):
    return np.float32(1.0) / (np.float32(1.0) + np.exp(-v))


def _unpack16(x, thr, results):
    """Device planes -> full [N*S*A, 6] f32 boxes with stable compaction."""
    yb = np.stack([np.asarray(r["y"]) for r in results])     # [8,M,P,3000] f16
    # [8, M, P, plane(5), a, j] -> row order (n, p, j, a) x field
    v = yb.reshape(NCORES, M, P, 5, A, J).transpose(0, 1, 2, 5, 4, 3)
    boxes5 = np.ascontiguousarray(v).reshape(N * S * A, 5).astype(np.float32)
    # plane order: 0=cx, 1=cy, 2=theta, 3=w, 4=h

    # patch rows where fp16 tanh near -1 cancels against the +1 grid/anchor
    # offset (ix=0 / iy=0 / a=0 with logit < -3): recompute exactly on host.
    for a in range(A):
        nn_, hh = np.nonzero(x[:, 1 + 6 * a, :, 0] < _PATCH_THR)
        boxes5[(nn_ * S + hh * W) * A + a, 0] = \
            32.0 * _sig(x[nn_, 1 + 6 * a, hh, 0])
        nn_, ww = np.nonzero(x[:, 2 + 6 * a, 0, :] < _PATCH_THR)
        boxes5[(nn_ * S + ww) * A + a, 1] = \
            32.0 * _sig(x[nn_, 2 + 6 * a, 0, ww])
    nn_, hh, ww = np.nonzero(x[:, 5] < _PATCH_THR)
    boxes5[(nn_ * S + hh * W + ww) * A, 2] = 60.0 * _sig(x[nn_, 5, hh, ww])

    logits = np.ascontiguousarray(
        x[:, 0::F, :, :].transpose(0, 2, 3, 1)
    ).reshape(-1)
    conf = _sig(logits)
    mask = conf >= np.float32(thr)
    k = int(mask.sum())
    sub = boxes5[mask]                                        # [k, 5]
    out = np.zeros((N * S * A, F), np.float32)
    out[:k, 0] = conf[mask]
    out[:k, 1] = sub[:, 0]
    out[:k, 2] = sub[:, 1]
    out[:k, 3] = sub[:, 3]
    out[:k, 4] = sub[:, 4]
    out[:k, 5] = sub[:, 2]
    return out


def _build_nc():
    """Build the per-core Bass module (same program on all 8 cores)."""
    import concourse.bacc as bacc
    import concourse.mybir as mybir
    import concourse.tile as tile

    f32 = mybir.dt.float32
    AF = mybir.ActivationFunctionType
    ALU = mybir.AluOpType

    nc = bacc.Bacc("TRN2", target_bir_lowering=False, debug=False)

    x = nc.dram_tensor("x", [M, C, H, W], f32, kind="ExternalInput")
    c1 = nc.dram_tensor("c1", [P, J], f32, kind="ExternalInput")
    c2 = nc.dram_tensor("c2", [P, J], f32, kind="ExternalInput")
    y = nc.dram_tensor("y", [M * S * A, F], f32, kind="ExternalOutput")

    # [M, C, S] view of the input; [M, P, 3600] view of the output where
    # partition p owns box rows [200p, 200p+200)*A of its image.
    xf = x.ap().rearrange("n c h w -> n c (h w)")
    yf = y.ap().rearrange("(n p q) f -> n p (q f)", n=M, p=P)

    ln_w = float(np.log(np.float32(ANCHOR_W)))
    ln_h = float(np.log(np.float32(ANCHOR_H)))

    with tile.TileContext(nc) as tc:
        with (
            tc.tile_pool(name="const", bufs=1) as constp,
            tc.tile_pool(name="inp", bufs=4) as inp,
            tc.tile_pool(name="outp", bufs=3) as outp,
            tc.tile_pool(name="tmp", bufs=2) as tmpp,
        ):
            c1_t = constp.tile([P, J], f32, tag="c1")
            nc.sync.dma_start(c1_t[:], c1.ap())
            c2_t = constp.tile([P, J], f32, tag="c2")
            nc.sync.dma_start(c2_t[:], c2.ap())
            bw_t = constp.tile([P, 1], f32, tag="bw")
            nc.vector.memset(bw_t[:], ln_w)
            bh_t = constp.tile([P, 1], f32, tag="bh")
            nc.vector.memset(bh_t[:], ln_h)
            # broadcast the [P, J] constants across the anchor dim
            c1v = c1_t[:].unsqueeze(1).broadcast_to([P, A, J])
            c2v = c2_t[:].unsqueeze(1).broadcast_to([P, A, J])

            def decode(inv, outv, outj, j0, j1):
                """Emit the 6 per-field pipelines for spatial cols [j0, j1)."""

                def tmp3(tag):
                    t = tmpp.tile([P, A * J], f32, tag=tag)
                    return t[:].rearrange("p (a j) -> p a j", a=A)[:, :, j0:j1]

                # f0: conf = 0.5 + 0.5*tanh(x/2)
                t0v = tmp3("t0")
                nc.scalar.activation(t0v, inv(0), AF.Tanh, scale=0.5)
                nc.vector.tensor_scalar(
                    out=outv(0), in0=t0v,
                    scalar1=0.5, scalar2=0.5, op0=ALU.mult, op1=ALU.add,
                )

                # f1: cx = (ix + sig)*32 = 16*(tanh + 2*ix + 1)
                t1v = tmp3("t1")
                nc.scalar.activation(t1v, inv(1), AF.Tanh, scale=0.5)
                u1v = tmp3("u1")
                nc.vector.tensor_add(u1v, t1v, c1v[:, :, j0:j1])
                nc.vector.tensor_scalar(
                    out=outv(1), in0=u1v, scalar1=16.0, scalar2=None,
                    op0=ALU.mult,
                )

                # f2: cy = 16*(tanh + 2*iy + 1)
                t2v = tmp3("t2")
                nc.scalar.activation(t2v, inv(2), AF.Tanh, scale=0.5)
                u2v = tmp3("u2")
                nc.vector.tensor_add(u2v, t2v, c2v[:, :, j0:j1])
                nc.vector.tensor_scalar(
                    out=outv(2), in0=u2v, scalar1=16.0, scalar2=None,
                    op0=ALU.mult,
                )

                # f3: w = exp(x + ln 85.72); f4: h = exp(x + ln 19.15)
                nc.scalar.activation(outv(3), inv(3), AF.Exp, bias=bw_t[:])
                nc.scalar.activation(outv(4), inv(4), AF.Exp, bias=bh_t[:])

                # f5: theta = (a + sig)*60 = 30*tanh + (60a + 30)
                t5v = tmp3("t5")
                nc.scalar.activation(t5v, inv(5), AF.Tanh, scale=0.5)
                for a in range(A):
                    nc.vector.tensor_scalar(
                        out=outj[:, F * a + 5, j0:j1],
                        in0=t5v[:, a],
                        scalar1=30.0, scalar2=60.0 * a + 30.0,
                        op0=ALU.mult, op1=ALU.add,
                    )

            for n in range(M):
                in_t = inp.tile([P, C * J], f32, tag="in")
                # channel c = a*6 + f sits at IN cols [c*J, (c+1)*J)
                invw = in_t[:].rearrange("p (a f j) -> p f a j", a=A, f=F)
                if n == 0:
                    # first image: per-field DMAs in pipeline order so the
                    # first ACT starts after 0.6MB instead of 1.84MB
                    for f in range(F):
                        nc.sync.dma_start(
                            invw[:, f],
                            xf[n].rearrange("(a f) (p j) -> f p a j",
                                            a=A, p=P)[f],
                        )
                else:
                    nc.sync.dma_start(
                        in_t[:].rearrange("p (c j) -> p c j", c=C),
                        xf[n].rearrange("c (p j) -> p c j", p=P),
                    )

                out_t = outp.tile([P, C * J], f32, tag="out")
                # OUT col = j*18 + a*6 + f  (row-major [76800, 6] boxes)
                outvw = out_t[:].rearrange("p (j a f) -> p f a j", a=A, f=F)
                outjw = out_t[:].rearrange("p (j c) -> p c j", c=C)

                halves = (0, J) if n < M - 1 else (0, J // 2, J)
                for h in range(len(halves) - 1):
                    j0, j1 = halves[h], halves[h + 1]
                    decode(lambda f: invw[:, f, :, j0:j1],
                           lambda f: outvw[:, f, :, j0:j1],
                           outjw, j0, j1)
                    # output rows for spatial cols [j0, j1) are contiguous
                    nc.sync.dma_start(
                        yf[n][:, j0 * C:j1 * C],
                        out_t[:, j0 * C:j1 * C],
                    )

    nc.compile()
    return nc


def _build_nc5():
    """Like _build_nc but the conf column is produced on the host (which
    already reads every conf logit for the compaction mask), so the device
    neither loads the 3 conf channels nor stores column 0: per-core traffic
    drops from 29.5MB to 24.6MB.

    Device output is the row-major [M*S*A, 5] matrix of (cx, cy, w, h, theta).
    """
    import concourse.bacc as bacc
    import concourse.mybir as mybir
    import concourse.tile as tile

    f32 = mybir.dt.float32
    AF = mybir.ActivationFunctionType
    ALU = mybir.AluOpType
    G = F - 1  # fields computed on device (1..5)

    nc = bacc.Bacc("TRN2", target_bir_lowering=False, debug=False)

    x = nc.dram_tensor("x", [M, C, H, W], f32, kind="ExternalInput")
    c1 = nc.dram_tensor("c1", [P, J], f32, kind="ExternalInput")
    c2 = nc.dram_tensor("c2", [P, J], f32, kind="ExternalInput")
    y = nc.dram_tensor("y", [M * S * A, G], f32, kind="ExternalOutput")

    xf = x.ap().rearrange("n c h w -> n c (h w)")
    yf = y.ap().rearrange("(n p q) f -> n p (q f)", n=M, p=P)

    ln_w = float(np.log(np.float32(ANCHOR_W)))
    ln_h = float(np.log(np.float32(ANCHOR_H)))

    with tile.TileContext(nc) as tc:
        with (
            tc.tile_pool(name="const", bufs=1) as constp,
            tc.tile_pool(name="inp", bufs=4) as inp,
            tc.tile_pool(name="outp", bufs=3) as outp,
            tc.tile_pool(name="tmp", bufs=2) as tmpp,
        ):
            c1_t = constp.tile([P, J], f32, tag="c1")
            nc.sync.dma_start(c1_t[:], c1.ap())
            c2_t = constp.tile([P, J], f32, tag="c2")
            nc.sync.dma_start(c2_t[:], c2.ap())
            bw_t = constp.tile([P, 1], f32, tag="bw")
            nc.vector.memset(bw_t[:], ln_w)
            bh_t = constp.tile([P, 1], f32, tag="bh")
            nc.vector.memset(bh_t[:], ln_h)
            c1v = c1_t[:].unsqueeze(1).broadcast_to([P, A, J])
            c2v = c2_t[:].unsqueeze(1).broadcast_to([P, A, J])

            def decode(inv, outv, outj, j0, j1):
                """fields 1..5 for spatial cols [j0, j1); conf is host-side."""

                def tmp3(tag):
                    t = tmpp.tile([P, A * J], f32, tag=tag)
                    return t[:].rearrange("p (a j) -> p a j", a=A)[:, :, j0:j1]

                # f1: cx = 16*(tanh + 2*ix + 1)
                t1v = tmp3("t1")
                nc.scalar.activation(t1v, inv(1), AF.Tanh, scale=0.5)
                u1v = tmp3("u1")
                nc.vector.tensor_add(u1v, t1v, c1v[:, :, j0:j1])
                nc.vector.tensor_scalar(
                    out=outv(1), in0=u1v, scalar1=16.0, scalar2=None,
                    op0=ALU.mult,
                )
                # f2: cy = 16*(tanh + 2*iy + 1)
                t2v = tmp3("t2")
                nc.scalar.activation(t2v, inv(2), AF.Tanh, scale=0.5)
                u2v = tmp3("u2")
                nc.vector.tensor_add(u2v, t2v, c2v[:, :, j0:j1])
                nc.vector.tensor_scalar(
                    out=outv(2), in0=u2v, scalar1=16.0, scalar2=None,
                    op0=ALU.mult,
                )
                # f3: w = exp(x + ln 85.72); f4: h = exp(x + ln 19.15)
                nc.scalar.activation(outv(3), inv(3), AF.Exp, bias=bw_t[:])
                nc.scalar.activation(outv(4), inv(4), AF.Exp, bias=bh_t[:])
                # f5: theta = 30*tanh + (60a + 30)
                t5v = tmp3("t5")
                nc.scalar.activation(t5v, inv(5), AF.Tanh, scale=0.5)
                for a in range(A):
                    nc.vector.tensor_scalar(
                        out=outj[:, G * a + 4, j0:j1],
                        in0=t5v[:, a],
                        scalar1=30.0, scalar2=60.0 * a + 30.0,
                        op0=ALU.mult, op1=ALU.add,
                    )

            C17 = C - 1  # channels 1..17 (conf channel 0 skipped; 6/12 dead)
            for n in range(M):
                # IN tile holds channels 1..17 in native order: channel c at
                # col (c-1)*J; field f anchor a -> c-1 = 6a + f - 1
                in_t = inp.tile([P, C17 * J], f32, tag="in")
                inw = in_t[:].rearrange("p (c j) -> p c j", c=C17)
                if n == 0:
                    # ramp: per-field DMAs in pipeline order
                    for f in range(1, F):
                        nc.sync.dma_start(
                            inw[:, f - 1:f + 12:F],
                            xf[n].rearrange("(a ff) (p j) -> ff p a j",
                                            a=A, p=P)[f],
                        )
                else:
                    # one DMA per image over the affine channel range 1..17
                    nc.sync.dma_start(
                        inw, xf[n][1:C].rearrange("c (p j) -> p c j", p=P),
                    )
                invw = None  # field views come from inw below

                out_t = outp.tile([P, A * G * J], f32, tag="out")
                # OUT col = j*15 + a*5 + (f-1)  (row-major [76800, 5])
                outvw = out_t[:].rearrange("p (j a f) -> p f a j", a=A, f=G)
                outjw = out_t[:].rearrange("p (j c) -> p c j", c=A * G)

                halves = (0, J) if n < M - 1 else (0, J // 2, J)
                for h in range(len(halves) - 1):
                    j0, j1 = halves[h], halves[h + 1]
                    decode(lambda f: inw[:, f - 1:f + 12:F, j0:j1],
                           lambda f: outvw[:, f - 1, :, j0:j1],
                           outjw, j0, j1)
                    nc.sync.dma_start(
                        yf[n][:, j0 * A * G:j1 * A * G],
                        out_t[:, j0 * A * G:j1 * A * G],
                    )

    nc.compile()
    return nc


def _build_nc_raw():
    """Hand-scheduled raw-bass variant: no TileContext barriers/preamble.

    Engine split: sync issues all input DMAs (HWDGE), scalar runs the 6 ACT
    ops per image, vector the 8 DVE ops, gpsimd issues output DMAs (SWDGE).
    Cyclic buffers (4x in, 3x out, 2x tmp) guarded by cumulative semaphore
    thresholds: s_in/s_out count DMA completions (x16), s_act/s_dve count
    compute ops.
    """
    from contextlib import ExitStack

    import concourse.bass as bass
    import concourse.mybir as mybir

    f32 = mybir.dt.float32
    AF = mybir.ActivationFunctionType
    ALU = mybir.AluOpType

    nc = bass.Bass("TRN2", target_bir_lowering=False, debug=False)

    x = nc.dram_tensor("x", [M, C, H, W], f32, kind="ExternalInput")
    # consts packed into one tensor: cols [0:J)=2*ix+1, [J:2J)=2*iy+1,
    # [2J]=ln(ANCHOR_W), [2J+1]=ln(ANCHOR_H)
    cc = nc.dram_tensor("cc", [P, 2 * J + 2], f32, kind="ExternalInput")
    y = nc.dram_tensor("y", [M * S * A, F], f32, kind="ExternalOutput")

    xf = x.ap().rearrange("n c h w -> n c (h w)")
    yf = y.ap().rearrange("(n p q) f -> n p (q f)", n=M, p=P)

    NBUF_IN, NBUF_OUT, NBUF_T = 5, 3, 2

    with ExitStack() as ctx:
        in_t = [ctx.enter_context(nc.sbuf_tensor(f"in{i}", [P, C * J], f32))
                for i in range(NBUF_IN)]
        out_t = [ctx.enter_context(nc.sbuf_tensor(f"out{i}", [P, C * J], f32))
                 for i in range(NBUF_OUT)]
        # tmp tanh tiles per field (t0,t1,t2,t5) and u tiles, double buffered
        tmps = {}
        for nm in ("t0", "t1", "t2", "t5", "u1", "u2"):
            tmps[nm] = [
                ctx.enter_context(nc.sbuf_tensor(f"{nm}_{i}", [P, A * J], f32))
                for i in range(NBUF_T)
            ]
        cc_t = ctx.enter_context(nc.sbuf_tensor("cc_t", [P, 2 * J + 2], f32))
        # one sem per DMA "slot" so milestone waits are never contaminated by
        # partial increments of a concurrently-running DMA on the same sem
        s_cc = ctx.enter_context(nc.semaphore("s_cc"))
        s_if = [ctx.enter_context(nc.semaphore(f"s_if{f}")) for f in range(F)]
        s_ib = [ctx.enter_context(nc.semaphore(f"s_ib{i}"))
                for i in range(NBUF_IN)]
        s_ih = [ctx.enter_context(nc.semaphore(f"s_ih{i}"))
                for i in range(NBUF_IN)]
        s_ob = [ctx.enter_context(nc.semaphore(f"s_ob{i}"))
                for i in range(NBUF_OUT)]
        s_act = ctx.enter_context(nc.semaphore("s_act"))
        s_dve = ctx.enter_context(nc.semaphore("s_dve"))
        block = ctx.enter_context(nc.Block())

        c1v = cc_t.ap()[:, 0:J].unsqueeze(1).broadcast_to([P, A, J])
        c2v = cc_t.ap()[:, J:2 * J].unsqueeze(1).broadcast_to([P, A, J])
        bw = cc_t.ap()[:, 2 * J:2 * J + 1]
        bh = cc_t.ap()[:, 2 * J + 1:2 * J + 2]

        # ---- static schedule bookkeeping (python-side counters) ----
        # input thresholds: img0 per-field on s_if[f]; img n>=1 split into a
        # low half (sync/HWDGE -> s_ib[n%4]) and high half (gpsimd/SWDGE ->
        # s_ih[n%4]); SWDGE and HWDGE must not share a semaphore
        def in_thrs(n):  # [(sem, value), ...] for image n loaded (n >= 1)
            v = 16 * ((n - 1) // NBUF_IN + 1)
            return [(s_ib[n % NBUF_IN], v)]

        # ACT op order: per image f0,f1,f2,f3,f4,f5 (img7: two j-halves)
        # DVE op order: f0ts, f1tt, f1ts, f2tt, f2ts, th0, th1, th2
        act_done_img = {}   # act count after image n's reads of in_t done
        dve_done_img = {}   # dve count after image n's writes to out_t done
        act_half = {}       # (n, h) -> act count after that half
        dve_half = {}
        # consumption points of tmp tiles (for ACT WAR on t*):
        dve_t_consumed = {}  # (name, n) -> dve count when t_name[n%2] free

        act_c = 0
        dve_c = 0
        for n in range(M):
            halves = (0, J) if n < M - 1 else (0, J // 2, J)
            for h in range(len(halves) - 1):
                act_c += 6
                dve_c += 8
                act_half[(n, h)] = act_c
                dve_half[(n, h)] = dve_c
            act_done_img[n] = act_c
            dve_done_img[n] = dve_c
            for nm in ("t0", "t1", "t2", "t5"):
                dve_t_consumed[(nm, n)] = dve_c  # conservative: end of image

        # per-out-buffer cumulative thresholds on s_ob[n%3]
        out_buf_cum = [0] * NBUF_OUT
        out_done_buf = {}   # n -> s_ob[n%3] value after image n's outs land
        for n in range(M):
            ndma = 2 if n == M - 1 else 1
            out_buf_cum[n % NBUF_OUT] += 16 * ndma
            out_done_buf[n] = out_buf_cum[n % NBUF_OUT]

        def img0_f_dma(eng, f):
            iv = in_t[0].ap().rearrange("p (a ff j) -> p ff a j",
                                        a=A, ff=F)[:, f]
            eng.dma_start(
                iv, xf[0].rearrange("(a ff) (p j) -> ff p a j",
                                    a=A, p=P)[f],
            ).then_inc(s_if[f], 16)

        # ---- sync engine: all input DMAs (one HWDGE ring) ----
        @block.sync
        def _(sync):
            for f in range(F):
                img0_f_dma(sync, f)
            for n in range(1, M):
                if n >= NBUF_IN:
                    sync.wait_ge(s_act, act_done_img[n - NBUF_IN])
                sync.dma_start(
                    in_t[n % NBUF_IN].ap().rearrange("p (c j) -> p c j", c=C),
                    xf[n].rearrange("c (p j) -> p c j", p=P),
                ).then_inc(s_ib[n % NBUF_IN], 16)

        # ---- scalar engine: ACT ops + high-half input DMAs ----
        @block.scalar
        def _(scalar):
            # dummy ACTIVATE before any wait so walrus's ACT_TABLE_LOAD for
            # exp_and_others runs during the input ramp, not after it
            const0 = nc.const_aps.aps[(f32, 0.0)]
            nc.scalar.activation(
                tmps["t0"][0].ap()[:, 0:1], const0[:, 0:1], AF.Tanh)
            scalar.dma_start(cc_t.ap(), cc.ap()).then_inc(s_cc, 16)
            scalar.wait_ge(s_cc, 16)  # exp bias tiles
            for n in range(M):
                ib = n % NBUF_IN
                ob = n % NBUF_OUT
                tb = n % NBUF_T
                invw = in_t[ib].ap().rearrange("p (a f j) -> p f a j",
                                               a=A, f=F)
                outvw = out_t[ob].ap().rearrange("p (j a f) -> p f a j",
                                                 a=A, f=F)
                halves = (0, J) if n < M - 1 else (0, J // 2, J)
                for h in range(len(halves) - 1):
                    j0, j1 = halves[h], halves[h + 1]
                    # data-ready wait
                    if n == 0:
                        pass  # per-f waits below
                    elif h == 0:
                        for sem, v in in_thrs(n):
                            scalar.wait_ge(sem, v)
                    # out_t WAR (f3/f4 write it)
                    if n >= NBUF_OUT and h == 0:
                        scalar.wait_ge(s_ob[n % NBUF_OUT],
                                       out_done_buf[n - NBUF_OUT])
                    # tmp WAR vs DVE of image n-2
                    if n >= NBUF_T and h == 0:
                        scalar.wait_ge(s_dve, dve_done_img[n - NBUF_T])

                    def tv(nm):
                        return tmps[nm][tb].ap().rearrange(
                            "p (a j) -> p a j", a=A)[:, :, j0:j1]

                    for f, func in ((0, AF.Tanh), (1, AF.Tanh), (2, AF.Tanh),
                                    (3, AF.Exp), (4, AF.Exp), (5, AF.Tanh)):
                        if n == 0:
                            scalar.wait_ge(s_if[f], 16)
                        iv = invw[:, f, :, j0:j1]
                        if func is AF.Exp:
                            b = bw if f == 3 else bh
                            inst = nc.scalar.activation(
                                outvw[:, f, :, j0:j1], iv, AF.Exp, bias=b)
                        else:
                            inst = nc.scalar.activation(
                                tv(f"t{f}" if f != 5 else "t5"), iv,
                                AF.Tanh, scale=0.5)
                        inst.then_inc(s_act, 1)

        # ---- vector engine: DVE ops ----
        @block.vector
        def _(vector):
            vector.wait_ge(s_cc, 16)  # consts loaded
            dve_c = 0
            u_read = {}  # (name, n) -> dve count after last read of u[name]
            for n in range(M):
                ob = n % NBUF_OUT
                tb = n % NBUF_T
                outvw = out_t[ob].ap().rearrange("p (j a f) -> p f a j",
                                                 a=A, f=F)
                outjw = out_t[ob].ap().rearrange("p (j c) -> p c j", c=C)
                halves = (0, J) if n < M - 1 else (0, J // 2, J)
                for h in range(len(halves) - 1):
                    j0, j1 = halves[h], halves[h + 1]
                    base_act = act_half[(n, h)] - 6

                    if n >= NBUF_OUT and h == 0:
                        vector.wait_ge(s_ob[n % NBUF_OUT],
                                       out_done_buf[n - NBUF_OUT])

                    def tv(nm):
                        return tmps[nm][tb].ap().rearrange(
                            "p (a j) -> p a j", a=A)[:, :, j0:j1]

                    # f0 conf
                    vector.wait_ge(s_act, base_act + 1)
                    nc.vector.tensor_scalar(
                        out=outvw[:, 0, :, j0:j1], in0=tv("t0"),
                        scalar1=0.5, scalar2=0.5,
                        op0=ALU.mult, op1=ALU.add,
                    ).then_inc(s_dve, 1)
                    dve_c += 1
                    # f1 cx (same-engine RAW on u1 and WAR vs image n-2)
                    vector.wait_ge(s_act, base_act + 2)
                    if ("u1", n - NBUF_T) in u_read:
                        vector.wait_ge(s_dve, u_read[("u1", n - NBUF_T)])
                    nc.vector.tensor_add(
                        tv("u1"), tv("t1"), c1v[:, :, j0:j1],
                    ).then_inc(s_dve, 1)
                    dve_c += 1
                    vector.wait_ge(s_dve, dve_c)
                    nc.vector.tensor_scalar(
                        out=outvw[:, 1, :, j0:j1], in0=tv("u1"),
                        scalar1=16.0, scalar2=None, op0=ALU.mult,
                    ).then_inc(s_dve, 1)
                    dve_c += 1
                    u_read[("u1", n)] = dve_c
                    # f2 cy
                    vector.wait_ge(s_act, base_act + 3)
                    if ("u2", n - NBUF_T) in u_read:
                        vector.wait_ge(s_dve, u_read[("u2", n - NBUF_T)])
                    nc.vector.tensor_add(
                        tv("u2"), tv("t2"), c2v[:, :, j0:j1],
                    ).then_inc(s_dve, 1)
                    dve_c += 1
                    vector.wait_ge(s_dve, dve_c)
                    nc.vector.tensor_scalar(
                        out=outvw[:, 2, :, j0:j1], in0=tv("u2"),
                        scalar1=16.0, scalar2=None, op0=ALU.mult,
                    ).then_inc(s_dve, 1)
                    dve_c += 1
                    u_read[("u2", n)] = dve_c
                    # f5 theta
                    vector.wait_ge(s_act, base_act + 6)
                    for a in range(A):
                        nc.vector.tensor_scalar(
                            out=outjw[:, F * a + 5, j0:j1],
                            in0=tv("t5")[:, a],
                            scalar1=30.0, scalar2=60.0 * a + 30.0,
                            op0=ALU.mult, op1=ALU.add,
                        ).then_inc(s_dve, 1)
                        dve_c += 1

        # ---- gpsimd engine (SWDGE): output DMAs ----
        @block.gpsimd
        def _(gpsimd):
            for n in range(M):
                ob = n % NBUF_OUT
                halves = (0, J) if n < M - 1 else (0, J // 2, J)
                for h in range(len(halves) - 1):
                    j0, j1 = halves[h], halves[h + 1]
                    gpsimd.wait_ge(s_act, act_half[(n, h)])
                    gpsimd.wait_ge(s_dve, dve_half[(n, h)])
                    gpsimd.dma_start(
                        yf[n][:, j0 * C:j1 * C],
                        out_t[ob].ap()[:, j0 * C:j1 * C],
                    ).then_inc(s_ob[ob], 16)
            for b in range(NBUF_OUT):
                gpsimd.wait_ge(s_ob[b], out_buf_cum[b])

    return nc


def _const_tiles():
    s = np.arange(S, dtype=np.int64).reshape(P, J)
    ix = (s % W).astype(np.float32)
    iy = (s // W).astype(np.float32)
    c1 = (2.0 * ix + 1.0).astype(np.float32)
    c2 = (2.0 * iy + 1.0).astype(np.float32)
    return np.ascontiguousarray(c1), np.ascontiguousarray(c2)


def _const_packed():
    c1, c2 = _const_tiles()
    ln_w = np.log(np.float32(ANCHOR_W)).astype(np.float32)
    ln_h = np.log(np.float32(ANCHOR_H)).astype(np.float32)
    tail = np.empty((P, 2), np.float32)
    tail[:, 0] = ln_w
    tail[:, 1] = ln_h
    return np.ascontiguousarray(np.concatenate([c1, c2, tail], axis=1))


def run(output, confidence_threshold, trace=False):
    """Run the kernel; returns (full_output, BassKernelResults)."""
    from concourse.bass_utils import run_bass_kernel_spmd

    x = np.asarray(output, dtype=np.float32)
    thr = float(np.asarray(confidence_threshold))
    assert x.shape == (N, C, H, W), x.shape

    import os
    impl = os.environ.get("DETECT_KERNEL_IMPL", "raw16b")
    builders = {"f16": _build_nc16, "raw16": _build_nc_raw16,
                "raw16b": _build_nc_raw16b, "raw16c": _build_nc_raw16c,
                "tile5": _build_nc5, "tile": _build_nc, "raw": _build_nc_raw}
    if impl not in _nc_cache:
        _nc_cache[impl] = builders[impl]()
    nc = _nc_cache[impl]

    if impl in ("f16", "raw16", "raw16b", "raw16c"):
        xp = _pack_inputs16(x)
        in_maps = [{"x": xp[d]} for d in range(NCORES)]
        if impl not in ("raw16b", "raw16c"):
            cc = _const16()
            for m_ in in_maps:
                m_["cc"] = cc
        res = run_bass_kernel_spmd(nc, in_maps, core_ids=list(range(NCORES)),
                                   trace=trace)
        if impl == "raw16c":
            return _unpack16c(x, thr, res.results), res
        return _unpack16(x, thr, res.results), res

    if impl == "raw":
        cc = _const_packed()
        in_maps = [
            {"x": np.ascontiguousarray(x[d * M:(d + 1) * M]), "cc": cc}
            for d in range(NCORES)
        ]
    else:
        c1, c2 = _const_tiles()
        in_maps = [
            {"x": np.ascontiguousarray(x[d * M:(d + 1) * M]),
             "c1": c1, "c2": c2}
            for d in range(NCORES)
        ]
    res = run_bass_kernel_spmd(nc, in_maps, core_ids=list(range(NCORES)),
                               trace=trace)
    boxes = np.concatenate([r["y"] for r in res.results], axis=0)

    # Stable compaction on host: valid rows (sigmoid(conf_logit) >= thr) first,
    # in original order; zero rows after. Mask from the raw logits in f32.
    logits = np.ascontiguousarray(
        x[:, 0::F, :, :].transpose(0, 2, 3, 1)
    ).reshape(-1)  # row order (n, h, w, a)
    conf = np.float32(1.0) / (np.float32(1.0) + np.exp(-logits))
    mask = conf >= np.float32(thr)
    k = int(mask.sum())
    out = np.zeros((N * S * A, F), np.float32)
    if impl == "tile5":
        # device produced (cx, cy, w, h, theta); conf column comes from the
        # same host sigmoid used for the mask
        out[:k, 0] = conf[mask]
        out[:k, 1:] = boxes[mask]
    else:
        out[:k] = boxes[mask]
    return out, res


def kernel(output, confidence_threshold):
    out, _ = run(output, confidence_threshold, trace=False)
    return out

